# revision 47
# baseline (speedup 1.0000x reference)
"""GRU-decoder kernel for 8 Trainium2 NeuronCores.

Math (all 127 output steps are identical — see the reference):
    x0   = relu(emb[input[:,0]])                       [B,H]
    h0   = einsum('blh,l->bh', hidden, bridge_w) + bb  [B,H]
    gi   = x0 @ w_ih.T + b_ih ; gh = h0 @ w_hh.T + b_hh
    r,z  = sigmoid(...) ; n = tanh(in + r*hn)
    h1   = (1-z)*n + z*h0
    logp = log_softmax(h1 @ proj_w.T + proj_b)         [B,V]
    out  = broadcast(logp, [B, L-1, V])

Sharding: vocab-parallel projection (each core owns V/8 rows of proj_w,
stored fp8e4 scaled x512, DoubleRow matmuls) plus h-sharded GRU (each
core owns a 128-wide slice of the hidden dim, computes partial gate
pre-activations, one slim bf16 AllReduce combines them).  True logits
are bounded (|logit| < ~6) so softmax runs without max subtraction; the
only global stat is sumexp, combined with a tiny AllGather + ones-matmul
reduction.  The [B,V] result is gathered on host and broadcast (a
zero-copy view) over the L-1 steps.

Scheduling notes:
  - sync HWDGE ring: 13 x 512KB fp8 weight-stream DMAs, then the
    post-projection stats/output DMAs (ring is idle by then).
  - scalar HWDGE ring: blob of small tensors (one DMA), hidden, GRU
    weights, collective bounce buffers.
  - activation tables (Sigmoid/Tanh/Exp/Ln) are preloaded with dummy
    ops during the initial DMA wait so no table load sits on the
    critical path.
  - a dummy 32B AllGather issued at t=0 warms up the collectives
    firmware before the real AllReduce.
"""

import numpy as np
import ml_dtypes

import concourse.bass as bass
import concourse.tile as tile
from concourse import bacc, mybir
from concourse.bass_utils import run_bass_kernel_spmd

B, L, H, V = 16, 128, 1024, 50257
NC = 8
VC = 6656                # per-core vocab shard (13*512); 8*VC = 53248 >= V
HC = H // NC             # per-core hidden-dim shard (128)
G3 = 3 * H               # gate rows (r,z,n)
NT = G3 // 128           # 24 j-tiles of 128
NG = VC // 512           # 13 projection column groups of 512
NEG = -1.0e30

WSCALE = 512.0           # proj_w scaled by 2^9 so fp8e4 values are normal
HSCALE = 16.0            # h1 scaled by 2^4 before fp8e4 cast
OSCALE = 1.0 / (WSCALE * HSCALE)   # logits de-scale: 2^-13

f32 = mybir.dt.float32
bf16 = mybir.dt.bfloat16
f8e4 = mybir.dt.float8e4
FX = mybir.ActivationFunctionType
AX = mybir.AxisListType
ALU = mybir.AluOpType
DR = mybir.MatmulPerfMode.DoubleRow

NP_F8E4 = ml_dtypes.float8_e4m3
NP_BF16 = ml_dtypes.bfloat16

# blob column layout (f32, [128, 74]):
#   0:16 x0T | 16:40 biT | 40:64 bhT | 64 bw | 65 bb | 66:74 msk
BLOB_C = 74

# AllReduce payload tiles (each [128, tile, B] bf16):
#   0:16 gi+gh partials for r,z | 16:24 in_ | 24:32 hn | 32:40 h0 masked
AR_T = 40

# tail split: vector handles the first TS_V cols, scalar engine the rest
TS_V = 8 * 512

LAST_RESULT = None  # test harness reads profiling info from here
_NC_CACHE = None


def _bc(ap, insert_at, step, count):
    """Insert a broadcast/strided dim into an AP at position insert_at."""
    new = list(ap.ap)
    new.insert(insert_at, [step, count])
    return bass.AP(tensor=ap.tensor, offset=ap.offset, ap=new)


def _build():
    nc = bacc.Bacc("TRN2", target_bir_lowering=False, debug=False, num_devices=NC)

    blob = nc.dram_tensor("blob", [128, BLOB_C], f32, kind="ExternalInput").ap()
    # hid carries [hid | wihT | whhT] packed as one [128, 8192] bf16 tensor
    hid = nc.dram_tensor("hid", [L, 2048 + 2 * G3], bf16, kind="ExternalInput").ap()
    pw8 = nc.dram_tensor("pw8", [128, NG * 8 * 512], f8e4, kind="ExternalInput").ap()
    pb = nc.dram_tensor("pb", [1, VC], f32, kind="ExternalInput").ap()
    logp = nc.dram_tensor("logp", [B, VC], bf16, kind="ExternalOutput").ap()

    ex1 = nc.alloc_semaphore("ex1")   # gates exchange: peers' payload arrived
    loc1 = nc.alloc_semaphore("loc1")
    ex2 = nc.alloc_semaphore("ex2")   # stats exchange
    loc2 = nc.alloc_semaphore("loc2")
    prep1 = nc.alloc_semaphore("prep1")  # descriptor-write completion gates
    prep2 = nc.alloc_semaphore("prep2")

    with tile.TileContext(nc) as tc:
        with (
            tc.tile_pool(name="singles", bufs=1) as singles,
            tc.tile_pool(name="gru_ps", bufs=1, space="PSUM") as gru_ps,
            tc.tile_pool(name="proj_ps", bufs=3, space="PSUM") as proj_ps,
            tc.tile_pool(name="gs_ps", bufs=1, space="PSUM") as gs_pool,
            tc.tile_pool(name="stats", bufs=4) as stats,
        ):
            # ---- critical loads first on the sync HWDGE ring --------------
            # one [128, 8192] bf16 transfer carries hid | wihT | whhT; views
            # are hand-built APs into it (one DMA fixed cost instead of 3)
            ebuf = singles.tile([128, 2048 + 2 * G3], bf16, tag="ebuf")
            nc.sync.dma_start(out=ebuf, in_=hid)
            hid_sb = ebuf[:, 0:2048].rearrange("p (b h) -> p b h", b=B)
            wih_sb = ebuf[:, 2048 : 2048 + G3]
            whh_sb = ebuf[:, 2048 + G3 : 2048 + 2 * G3]

            # small/noncritical on the scalar ring
            blob_sb = singles.tile([128, BLOB_C], f32, tag="blob_sb")
            nc.scalar.dma_start(out=blob_sb, in_=blob)
            pbb = singles.tile([B, VC], f32, tag="pbb")
            nc.scalar.dma_start(out=pbb, in_=_bc(pb[0], 0, 0, B))

            # ---- fp8 weight stream (sync ring, after the critical loads;
            # 13 moderate transfers keep SDMA packet turns short) ----------
            # host layout per partition: [g(13), kc2(4), sub(2), j(2), c(256)]
            pw_sb = singles.tile([128, NG, 4, 2, 2, 256], f8e4, tag="pw_sb")
            pw_view = pw8.rearrange("p (g x) -> p g x", g=NG)
            for g in range(NG):
                nc.sync.dma_start(out=pw_sb[:, g], in_=pw_view[:, g])

            # ---- unpack blob ----------------------------------------------
            x0T_sb = singles.tile([HC, B], bf16, tag="x0T_sb")
            nc.scalar.activation(out=x0T_sb[:], in_=blob_sb[:, 0:16], func=FX.Relu)
            bw_sb = singles.tile([L, 1], bf16, tag="bw_sb")
            nc.vector.tensor_copy(bw_sb[:], blob_sb[:, 64:65])
            msk_sb = singles.tile([128, NC], bf16, tag="msk_sb")
            nc.vector.tensor_copy(msk_sb[:], blob_sb[:, 66:74])
            bsum = singles.tile([128, 16], f32, tag="bsum")
            nc.vector.tensor_add(bsum, blob_sb[:, 16:32], blob_sb[:, 40:56])

            # ---- bridge: h0T_c[h,b] = sum_l hidden[b,l,h]*w[l] ------------
            h0T_ps = gru_ps.tile([HC, B], f32, tag="h0T_ps")
            for b in range(B):
                nc.tensor.matmul(
                    h0T_ps[:, b : b + 1], hid_sb[:, b, :], bw_sb[:],
                    start=True, stop=True,
                )
            h0T_sb = singles.tile([HC, B], bf16, tag="h0T_sb")
            nc.vector.tensor_scalar_add(h0T_sb[:], h0T_ps[:], blob_sb[:, 65:66])

            # ---- partial gate pre-activations (T layout) ------------------
            # r,z gates: gi+gh accumulated in one PSUM group; n gate split.
            grz_ps = gru_ps.tile([128, 16, B], f32, tag="grz_ps")
            gin_ps = gru_ps.tile([128, 8, B], f32, tag="gin_ps")
            ghn_ps = gru_ps.tile([128, 8, B], f32, tag="ghn_ps")
            for t in range(16):
                nc.tensor.matmul(
                    grz_ps[:, t, :], wih_sb[:, t * 128 : (t + 1) * 128], x0T_sb[:],
                    start=True, stop=False,
                )
                nc.tensor.matmul(
                    grz_ps[:, t, :], whh_sb[:, t * 128 : (t + 1) * 128], h0T_sb[:],
                    start=False, stop=True,
                )
            for t in range(16, NT):
                nc.tensor.matmul(
                    gin_ps[:, t - 16, :], wih_sb[:, t * 128 : (t + 1) * 128], x0T_sb[:],
                    start=True, stop=True,
                )
                nc.tensor.matmul(
                    ghn_ps[:, t - 16, :], whh_sb[:, t * 128 : (t + 1) * 128], h0T_sb[:],
                    start=True, stop=True,
                )

            # ---- pack slim exchange payload [128, 40, 16] bf16 ------------
            arbuf = singles.tile([128, AR_T, B], bf16, tag="arbuf")
            nc.vector.tensor_copy(arbuf[:, 0:16, :], grz_ps[:])
            nc.vector.tensor_copy(arbuf[:, 16:24, :], gin_ps[:])
            nc.vector.tensor_copy(arbuf[:, 24:32, :], ghn_ps[:])
            h0_bcast = _bc(h0T_sb[:], 1, 0, NC)          # [128, 8, 16]
            msk_bcast = _bc(msk_sb[:], 2, 0, B)          # [128, 8, 16]
            nc.vector.tensor_mul(arbuf[:, 32:40, :], h0_bcast, msk_bcast)

            # ---- all-to-all exchange of partials (no ncfw, XOR slots) -----
            # broadcast #k sends my payload to core (me XOR k), into static
            # slot k.  Receiver d's slot k holds sender d^k — a permutation,
            # and the summed reduction is order-invariant.  Slot 0 (self) is
            # filled by a local copy.  Descriptor preps encode addresses
            # only, so they are dep-decoupled (decoy tracking offset) and
            # run early; the single trigger is gated on the packed payload
            # through a tiny gpsimd op that genuinely reads arbuf.
            arx_all = singles.tile([128, NC, AR_T * B], bf16, tag="arx_all")
            ab = arbuf[:]
            ab_flat = bass.AP(tensor=ab.tensor, offset=ab.offset,
                              ap=[ab.ap[0], [1, AR_T * B]])
            ab_decoy = bass.AP(tensor=ab.tensor, offset=ab.offset,
                               ap=[ab.ap[0], [1, AR_T * B]],
                               dep_tracking_offset=ab.offset + 2 * AR_T * B)
            for k in range(1, NC):
                nc.gpsimd.remote_dma_broadcast(
                    out_ap=arx_all[:, k, :],
                    in_ap=ab_decoy,
                    remote_sem=ex1, local_sem=loc1,
                    rdests=[(0, k) if j == k else None for j in range(NC)],
                )
            nc.vector.tensor_copy(arx_all[:, 0, :], ab_flat)
            gate1 = stats.tile([1, 1], bf16, tag="gate1")
            nc.gpsimd.tensor_copy(gate1, arbuf[0:1, 0:1, 0:1])
            nc.gpsimd.trigger_dma(count=NC - 1)
            # preload sigmoid/tanh tables while the exchange is in flight
            tl = stats.tile([128, 1], f32, tag="tl")
            nc.scalar.activation(out=tl, in_=arbuf[:, 0:1, 0:1], func=FX.Tanh)
            nc.scalar.activation(out=tl, in_=tl, func=FX.Sigmoid)
            # the ex1>=14 wait (7 remote senders x 2 increments) is attached
            # to i_red1 AFTER tile scheduling — the single-core scheduling
            # sim can't see remote increments and would declare deadlock
            arxv = arx_all[:].rearrange("p c (t b) -> p c t b", t=AR_T)
            arx = singles.tile([128, AR_T, B], f32, tag="arx")
            i_red1 = nc.vector.tensor_add(arx[:], arxv[:, 0], arxv[:, 1])
            for k in range(2, NC):
                nc.vector.tensor_add(arx[:], arx[:], arxv[:, k])

            # ---- gates (full width, every core redundantly) ---------------
            rT = singles.tile([128, NC, B], f32, tag="rT")
            nc.vector.tensor_add(rT[:], arx[:, 0:8, :], _bc(bsum[:, 0:8], 2, 0, B))
            nc.scalar.activation(out=rT[:], in_=rT[:], func=FX.Sigmoid)

            zT = singles.tile([128, NC, B], f32, tag="zT")
            nc.vector.tensor_add(zT[:], arx[:, 8:16, :], _bc(bsum[:, 8:16], 2, 0, B))
            nc.scalar.activation(out=zT[:], in_=zT[:], func=FX.Sigmoid)

            nT = singles.tile([128, NC, B], f32, tag="nT")
            nc.vector.tensor_add(nT[:], arx[:, 24:32, :], _bc(blob_sb[:, 56:64], 2, 0, B))
            nc.vector.tensor_mul(nT[:], nT[:], rT[:])
            nc.vector.tensor_add(nT[:], nT[:], arx[:, 16:24, :])
            nc.vector.tensor_add(nT[:], nT[:], _bc(blob_sb[:, 32:40], 2, 0, B))
            nc.scalar.activation(out=nT[:], in_=nT[:], func=FX.Tanh)

            h1T = singles.tile([128, NC, B], f32, tag="h1T")
            nc.vector.tensor_mul(h1T[:], zT[:], arx[:, 32:40, :])   # z*h0
            nc.vector.tensor_mul(zT[:], zT[:], nT[:])               # z*n
            nc.vector.tensor_add(h1T[:], h1T[:], nT[:])             # + n
            nc.vector.tensor_sub(h1T[:], h1T[:], zT[:])             # - z*n
            h1q = singles.tile([128, NC, B], f8e4, tag="h1q")
            nc.vector.tensor_scalar_mul(h1q[:], h1T[:], HSCALE)

            # ---- projection (DoubleRow fp8) + online sumexp ---------------
            logits_sb = singles.tile([B, VC], f32, tag="logits_sb")
            s_run = singles.tile([B, 1], f32, tag="s_run")
            nc.vector.memset(s_run, 0.0)

            for g in range(NG):
                col = g * 512
                lg = proj_ps.tile([B, 512], f32, tag="lg")
                for sub in range(2):
                    for k2 in range(4):
                        nc.tensor.matmul(
                            lg[:, sub * 256 : (sub + 1) * 256],
                            h1q[:, 2 * k2 : 2 * k2 + 2, :],
                            pw_sb[:, g, k2, sub],
                            start=(k2 == 0), stop=(k2 == 3),
                            perf_mode=DR,
                        )
                nc.vector.tensor_add(
                    logits_sb[:, col : col + 512], lg[:], pbb[:, col : col + 512]
                )
                expb = stats.tile([B, 512], f32, tag="expb")
                csum = stats.tile([B, 1], f32, tag="csum")
                nc.scalar.activation(
                    out=expb[:], in_=logits_sb[:, col : col + 512], func=FX.Exp,
                    scale=OSCALE, accum_out=csum[:, 0:1],
                )
                nc.vector.tensor_add(s_run, s_run, csum)

            # ---- global sumexp via the second XOR-slot exchange -----------
            sstage = singles.tile([128, 1], f32, tag="sstage")
            nc.vector.tensor_copy(sstage[0:B, :], s_run[:])
            s_all = singles.tile([128, NC], f32, tag="s_all")
            nc.vector.tensor_copy(s_all[:, 0:1], sstage[:])
            # these preps keep their real dep on sstage so they enter the
            # SWDGE ring strictly after the gates-exchange trigger
            for k in range(1, NC):
                nc.gpsimd.remote_dma_broadcast(
                    out_ap=s_all[:, k : k + 1],
                    in_ap=sstage[:],
                    remote_sem=ex2, local_sem=loc2,
                    rdests=[(0, k) if j == k else None for j in range(NC)],
                )
            # padding ops: keep the trigger from racing the last prep on the
            # second Q7 core (desc-gen ~0.9us; each pad ~0.3us dispatch)
            gate2 = stats.tile([1, 1], f32, tag="gate2")
            for _ in range(4):
                nc.gpsimd.tensor_copy(gate2, sstage[0:1, 0:1])
            nc.gpsimd.trigger_dma(count=NC - 1)
            # preload Ln + Identity tables while the exchange is in flight
            tl2 = stats.tile([B, 1], f32, tag="tl2")
            nc.scalar.activation(out=tl2, in_=s_run[:], func=FX.Ln)
            nc.scalar.activation(out=tl2, in_=tl2, func=FX.Identity, scale=1.0, bias=0.0)
            gS = singles.tile([B, 1], f32, tag="gS")
            i_red2 = nc.vector.reduce_sum(gS, s_all[0:B, :], axis=AX.X)
            ngS = singles.tile([B, 1], f32, tag="ngS")
            nc.scalar.activation(out=ngS, in_=gS[:], func=FX.Ln)
            nc.vector.tensor_scalar_mul(ngS, ngS, -1.0)    # -lse

            # ---- logp = logits*OSCALE - lse, write out (split engines,
            # bf16 output halves writeback bytes; host upcasts) ------------
            lp16 = singles.tile([B, VC], bf16, tag="lp16")
            nc.vector.tensor_scalar(
                out=lp16[:, 0:TS_V], in0=logits_sb[:, 0:TS_V],
                scalar1=OSCALE, scalar2=ngS[:, 0:1], op0=ALU.mult, op1=ALU.add,
            )
            nc.scalar.activation(
                out=lp16[:, TS_V:VC], in_=logits_sb[:, TS_V:VC],
                func=FX.Identity, scale=OSCALE, bias=ngS[:, 0:1],
            )
            nc.sync.dma_start(out=logp[:, 0:TS_V], in_=lp16[:, 0:TS_V])
            nc.sync.dma_start(out=logp[:, TS_V:VC], in_=lp16[:, TS_V:VC])

    # attach the cross-core arrival waits post-scheduling: 7 remote senders
    # x 2 sem increments each (the single-core scheduling sim would deadlock
    # on these since it can't model remote increments)
    i_red1.wait_op(ex1, 14, "sem-ge", check=False)
    i_red2.wait_op(ex2, 14, "sem-ge", check=False)

    nc.compile()
    return nc


def kernel(input, hidden, emb, bridge_w, bridge_b, w_ih, w_hh, b_ih, b_hh,
           proj_w, proj_b):
    global _NC_CACHE, LAST_RESULT
    if _NC_CACHE is None:
        _NC_CACHE = _build()
    nc = _NC_CACHE

    input = np.asarray(input)
    hidden = np.asarray(hidden, dtype=np.float32)
    emb = np.asarray(emb, dtype=np.float32)
    bridge_w = np.asarray(bridge_w, dtype=np.float32)
    bridge_b = np.asarray(bridge_b, dtype=np.float32)
    w_ih = np.asarray(w_ih, dtype=np.float32)
    w_hh = np.asarray(w_hh, dtype=np.float32)
    b_ih = np.asarray(b_ih, dtype=np.float32)
    b_hh = np.asarray(b_hh, dtype=np.float32)
    proj_w = np.asarray(proj_w, dtype=np.float32)
    proj_b = np.asarray(proj_b, dtype=np.float32)

    x0 = emb[input[:, 0].astype(np.int64)]          # [B, H]
    x0T = np.ascontiguousarray(x0.T)                # [H, B] f32 (relu on device)
    hid_t = np.ascontiguousarray(hidden.transpose(1, 0, 2)).astype(NP_BF16)

    biT = np.ascontiguousarray(b_ih.reshape(NT, 128).T)   # [128, 24]
    bhT = np.ascontiguousarray(b_hh.reshape(NT, 128).T)

    in_maps = []
    for c in range(NC):
        hs = slice(c * HC, (c + 1) * HC)
        lo, hi = c * VC, min((c + 1) * VC, V)
        pw_blk = proj_w[lo:hi]
        pb_blk = proj_b[lo:hi]
        if hi - lo < VC:
            pad = VC - (hi - lo)
            pw_blk = np.concatenate([pw_blk, np.zeros((pad, H), np.float32)], axis=0)
            pb_blk = np.concatenate([pb_blk, np.full((pad,), NEG, np.float32)])
        # DoubleRow layout: h = kc2*256 + j*128 + p ; v = g*512 + sub*256 + cc
        pwT = np.ascontiguousarray(pw_blk.T) * WSCALE          # [H, VC]
        pw_i = pwT.reshape(4, 2, 128, NG, 2, 256)              # [kc2,j,p,g,sub,c]
        pw_i = pw_i.transpose(2, 3, 0, 4, 1, 5)                # [p,g,kc2,sub,j,c]
        pw_i = np.ascontiguousarray(pw_i).reshape(128, NG * 8 * 256 * 2 // 512 * 512)

        blob = np.zeros((128, BLOB_C), np.float32)
        blob[:, 0:16] = x0T[hs]
        blob[:, 16:40] = biT
        blob[:, 40:64] = bhT
        blob[:, 64] = bridge_w[0]
        blob[:, 65] = bridge_b[0]
        blob[:, 66 + c] = 1.0                                  # mask one-hot

        ebuf = np.concatenate([
            hid_t[:, :, hs].reshape(L, B * HC).astype(NP_BF16),
            np.ascontiguousarray(w_ih[:, hs].T).astype(NP_BF16),
            np.ascontiguousarray(w_hh[:, hs].T).astype(NP_BF16),
        ], axis=1)
        in_maps.append({
            "blob": blob,
            "hid": np.ascontiguousarray(ebuf),
            "pw8": pw_i.astype(NP_F8E4),
            "pb": np.ascontiguousarray((pb_blk * WSCALE).reshape(1, VC)),
        })

    res = run_bass_kernel_spmd(nc, in_maps, list(range(NC)))
    LAST_RESULT = res

    logp_full = np.concatenate(
        [res.results[c]["logp"].astype(np.float32) for c in range(NC)], axis=1
    )
    logp_full = np.ascontiguousarray(logp_full[:, :V])
    return np.broadcast_to(logp_full[:, None, :], (B, L - 1, V))


# revision 48
# speedup vs baseline: 50.8941x; 50.8941x over previous
"""GRU-decoder kernel for 8 Trainium2 NeuronCores.

Math (all 127 output steps are identical — see the reference):
    x0   = relu(emb[input[:,0]])                       [B,H]
    h0   = einsum('blh,l->bh', hidden, bridge_w) + bb  [B,H]
    gi   = x0 @ w_ih.T + b_ih ; gh = h0 @ w_hh.T + b_hh
    r,z  = sigmoid(...) ; n = tanh(in + r*hn)
    h1   = (1-z)*n + z*h0
    logp = log_softmax(h1 @ proj_w.T + proj_b)         [B,V]
    out  = broadcast(logp, [B, L-1, V])

Sharding: vocab-parallel projection (each core owns V/8 rows of proj_w,
stored fp8e4 scaled x512, DoubleRow matmuls) plus h-sharded GRU (each
core owns a 128-wide slice of the hidden dim, computes partial gate
pre-activations, one slim bf16 AllReduce combines them).  True logits
are bounded (|logit| < ~6) so softmax runs without max subtraction; the
only global stat is sumexp, combined with a tiny AllGather + ones-matmul
reduction.  The [B,V] result is gathered on host and broadcast (a
zero-copy view) over the L-1 steps.

Scheduling notes:
  - sync HWDGE ring: 13 x 512KB fp8 weight-stream DMAs, then the
    post-projection stats/output DMAs (ring is idle by then).
  - scalar HWDGE ring: blob of small tensors (one DMA), hidden, GRU
    weights, collective bounce buffers.
  - activation tables (Sigmoid/Tanh/Exp/Ln) are preloaded with dummy
    ops during the initial DMA wait so no table load sits on the
    critical path.
  - a dummy 32B AllGather issued at t=0 warms up the collectives
    firmware before the real AllReduce.
"""

import numpy as np
import ml_dtypes

import concourse.bass as bass
import concourse.tile as tile
from concourse import bacc, mybir
from concourse.bass_utils import run_bass_kernel_spmd

B, L, H, V = 16, 128, 1024, 50257
NC = 8
VC = 6656                # per-core vocab shard (13*512); 8*VC = 53248 >= V
HC = H // NC             # per-core hidden-dim shard (128)
G3 = 3 * H               # gate rows (r,z,n)
NT = G3 // 128           # 24 j-tiles of 128
NG = VC // 512           # 13 projection column groups of 512
NEG = -1.0e30

WSCALE = 512.0           # proj_w scaled by 2^9 so fp8e4 values are normal
HSCALE = 16.0            # h1 scaled by 2^4 before fp8e4 cast
OSCALE = 1.0 / (WSCALE * HSCALE)   # logits de-scale: 2^-13

f32 = mybir.dt.float32
bf16 = mybir.dt.bfloat16
f8e4 = mybir.dt.float8e4
FX = mybir.ActivationFunctionType
AX = mybir.AxisListType
ALU = mybir.AluOpType
DR = mybir.MatmulPerfMode.DoubleRow

NP_F8E4 = ml_dtypes.float8_e4m3
NP_BF16 = ml_dtypes.bfloat16

# blob column layout (f32, [128, 74]):
#   0:16 x0T | 16:40 biT | 40:64 bhT | 64 bw | 65 bb | 66:74 msk
BLOB_C = 74

# AllReduce payload tiles (each [128, tile, B] bf16):
#   0:16 gi+gh partials for r,z | 16:24 in_ | 24:32 hn | 32:40 h0 masked
AR_T = 40

# tail split: vector handles the first TS_V cols, scalar engine the rest
TS_V = 8 * 512

LAST_RESULT = None  # test harness reads profiling info from here
_NC_CACHE = None


def _bc(ap, insert_at, step, count):
    """Insert a broadcast/strided dim into an AP at position insert_at."""
    new = list(ap.ap)
    new.insert(insert_at, [step, count])
    return bass.AP(tensor=ap.tensor, offset=ap.offset, ap=new)


def _build():
    nc = bacc.Bacc("TRN2", target_bir_lowering=False, debug=False, num_devices=NC)

    blob = nc.dram_tensor("blob", [128, BLOB_C], f32, kind="ExternalInput").ap()
    # hid carries [hid | wihT | whhT] packed as one [128, 8192] bf16 tensor
    hid = nc.dram_tensor("hid", [L, 2048 + 2 * G3], bf16, kind="ExternalInput").ap()
    pw8 = nc.dram_tensor("pw8", [128, NG * 8 * 512], f8e4, kind="ExternalInput").ap()
    pb = nc.dram_tensor("pb", [1, VC], f32, kind="ExternalInput").ap()
    logp = nc.dram_tensor("logp", [B, VC], bf16, kind="ExternalOutput").ap()

    ex1 = nc.alloc_semaphore("ex1")   # gates exchange: peers' payload arrived
    loc1 = nc.alloc_semaphore("loc1")
    ex2 = nc.alloc_semaphore("ex2")   # stats exchange
    loc2 = nc.alloc_semaphore("loc2")
    prep1 = nc.alloc_semaphore("prep1")  # descriptor-write completion gates
    prep2 = nc.alloc_semaphore("prep2")

    with tile.TileContext(nc) as tc:
        with (
            tc.tile_pool(name="singles", bufs=1) as singles,
            tc.tile_pool(name="gru_ps", bufs=1, space="PSUM") as gru_ps,
            tc.tile_pool(name="proj_ps", bufs=3, space="PSUM") as proj_ps,
            tc.tile_pool(name="gs_ps", bufs=1, space="PSUM") as gs_pool,
            tc.tile_pool(name="stats", bufs=4) as stats,
            tc.tile_pool(name="dram", bufs=1, space="DRAM") as dram,
        ):
            # ---- token ncfw collective: a NEFF containing a collective is
            # group-launched by the runtime (~20us core stagger); without
            # one the 8 cores dispatch serially ~1.1ms apart.  Gathers
            # garbage, output unused, never on the critical path.
            wcc_in = dram.tile([1, 8], f32, tag="wcc_in")
            wcc_out = dram.tile([NC, 8], f32, tag="wcc_out")
            nc.gpsimd.collective_compute(
                "AllGather", ALU.bypass,
                replica_groups=[list(range(NC))],
                ins=[wcc_in.opt()], outs=[wcc_out.opt()],
            )

            # ---- critical loads first on the sync HWDGE ring --------------
            # one [128, 8192] bf16 transfer carries hid | wihT | whhT; views
            # are hand-built APs into it (one DMA fixed cost instead of 3)
            ebuf = singles.tile([128, 2048 + 2 * G3], bf16, tag="ebuf")
            nc.sync.dma_start(out=ebuf, in_=hid)
            hid_sb = ebuf[:, 0:2048].rearrange("p (b h) -> p b h", b=B)
            wih_sb = ebuf[:, 2048 : 2048 + G3]
            whh_sb = ebuf[:, 2048 + G3 : 2048 + 2 * G3]

            # small/noncritical on the scalar ring
            blob_sb = singles.tile([128, BLOB_C], f32, tag="blob_sb")
            nc.scalar.dma_start(out=blob_sb, in_=blob)
            pbb = singles.tile([B, VC], f32, tag="pbb")
            nc.scalar.dma_start(out=pbb, in_=_bc(pb[0], 0, 0, B))

            # ---- fp8 weight stream (sync ring, after the critical loads;
            # 13 moderate transfers keep SDMA packet turns short) ----------
            # host layout per partition: [g(13), kc2(4), sub(2), j(2), c(256)]
            pw_sb = singles.tile([128, NG, 4, 2, 2, 256], f8e4, tag="pw_sb")
            pw_view = pw8.rearrange("p (g x) -> p g x", g=NG)
            for g in range(NG):
                nc.sync.dma_start(out=pw_sb[:, g], in_=pw_view[:, g])

            # ---- unpack blob ----------------------------------------------
            x0T_sb = singles.tile([HC, B], bf16, tag="x0T_sb")
            nc.scalar.activation(out=x0T_sb[:], in_=blob_sb[:, 0:16], func=FX.Relu)
            bw_sb = singles.tile([L, 1], bf16, tag="bw_sb")
            nc.vector.tensor_copy(bw_sb[:], blob_sb[:, 64:65])
            msk_sb = singles.tile([128, NC], bf16, tag="msk_sb")
            nc.vector.tensor_copy(msk_sb[:], blob_sb[:, 66:74])
            bsum = singles.tile([128, 16], f32, tag="bsum")
            nc.vector.tensor_add(bsum, blob_sb[:, 16:32], blob_sb[:, 40:56])

            # ---- bridge: h0T_c[h,b] = sum_l hidden[b,l,h]*w[l] ------------
            h0T_ps = gru_ps.tile([HC, B], f32, tag="h0T_ps")
            for b in range(B):
                nc.tensor.matmul(
                    h0T_ps[:, b : b + 1], hid_sb[:, b, :], bw_sb[:],
                    start=True, stop=True,
                )
            h0T_sb = singles.tile([HC, B], bf16, tag="h0T_sb")
            nc.vector.tensor_scalar_add(h0T_sb[:], h0T_ps[:], blob_sb[:, 65:66])

            # ---- partial gate pre-activations (T layout) ------------------
            # r,z gates: gi+gh accumulated in one PSUM group; n gate split.
            grz_ps = gru_ps.tile([128, 16, B], f32, tag="grz_ps")
            gin_ps = gru_ps.tile([128, 8, B], f32, tag="gin_ps")
            ghn_ps = gru_ps.tile([128, 8, B], f32, tag="ghn_ps")
            for t in range(16):
                nc.tensor.matmul(
                    grz_ps[:, t, :], wih_sb[:, t * 128 : (t + 1) * 128], x0T_sb[:],
                    start=True, stop=False,
                )
                nc.tensor.matmul(
                    grz_ps[:, t, :], whh_sb[:, t * 128 : (t + 1) * 128], h0T_sb[:],
                    start=False, stop=True,
                )
            for t in range(16, NT):
                nc.tensor.matmul(
                    gin_ps[:, t - 16, :], wih_sb[:, t * 128 : (t + 1) * 128], x0T_sb[:],
                    start=True, stop=True,
                )
                nc.tensor.matmul(
                    ghn_ps[:, t - 16, :], whh_sb[:, t * 128 : (t + 1) * 128], h0T_sb[:],
                    start=True, stop=True,
                )

            # ---- pack slim exchange payload [128, 40, 16] bf16 ------------
            arbuf = singles.tile([128, AR_T, B], bf16, tag="arbuf")
            nc.vector.tensor_copy(arbuf[:, 0:16, :], grz_ps[:])
            nc.vector.tensor_copy(arbuf[:, 16:24, :], gin_ps[:])
            nc.vector.tensor_copy(arbuf[:, 24:32, :], ghn_ps[:])
            h0_bcast = _bc(h0T_sb[:], 1, 0, NC)          # [128, 8, 16]
            msk_bcast = _bc(msk_sb[:], 2, 0, B)          # [128, 8, 16]
            nc.vector.tensor_mul(arbuf[:, 32:40, :], h0_bcast, msk_bcast)

            # ---- all-to-all exchange of partials (no ncfw, XOR slots) -----
            # broadcast #k sends my payload to core (me XOR k), into static
            # slot k.  Receiver d's slot k holds sender d^k — a permutation,
            # and the summed reduction is order-invariant.  Slot 0 (self) is
            # filled by a local copy.  Descriptor preps encode addresses
            # only, so they are dep-decoupled (decoy tracking offset) and
            # run early; the single trigger is gated on the packed payload
            # through a tiny gpsimd op that genuinely reads arbuf.
            arx_all = singles.tile([128, NC, AR_T * B], bf16, tag="arx_all")
            ab = arbuf[:]
            ab_flat = bass.AP(tensor=ab.tensor, offset=ab.offset,
                              ap=[ab.ap[0], [1, AR_T * B]])
            ab_decoy = bass.AP(tensor=ab.tensor, offset=ab.offset,
                               ap=[ab.ap[0], [1, AR_T * B]],
                               dep_tracking_offset=ab.offset + 2 * AR_T * B)
            for k in range(1, NC):
                nc.gpsimd.remote_dma_broadcast(
                    out_ap=arx_all[:, k, :],
                    in_ap=ab_decoy,
                    remote_sem=ex1, local_sem=loc1,
                    rdests=[(0, k) if j == k else None for j in range(NC)],
                )
            nc.vector.tensor_copy(arx_all[:, 0, :], ab_flat)
            gate1 = stats.tile([1, 1], bf16, tag="gate1")
            nc.gpsimd.tensor_copy(gate1, arbuf[0:1, 0:1, 0:1])
            nc.gpsimd.trigger_dma(count=NC - 1)
            # preload sigmoid/tanh tables while the exchange is in flight
            tl = stats.tile([128, 1], f32, tag="tl")
            nc.scalar.activation(out=tl, in_=arbuf[:, 0:1, 0:1], func=FX.Tanh)
            nc.scalar.activation(out=tl, in_=tl, func=FX.Sigmoid)
            # the ex1>=14 wait (7 remote senders x 2 increments) is attached
            # to i_red1 AFTER tile scheduling — the single-core scheduling
            # sim can't see remote increments and would declare deadlock
            arxv = arx_all[:].rearrange("p c (t b) -> p c t b", t=AR_T)
            arx = singles.tile([128, AR_T, B], f32, tag="arx")
            i_red1 = nc.vector.tensor_add(arx[:], arxv[:, 0], arxv[:, 1])
            for k in range(2, NC):
                nc.vector.tensor_add(arx[:], arx[:], arxv[:, k])

            # ---- gates (full width, every core redundantly) ---------------
            rT = singles.tile([128, NC, B], f32, tag="rT")
            nc.vector.tensor_add(rT[:], arx[:, 0:8, :], _bc(bsum[:, 0:8], 2, 0, B))
            nc.scalar.activation(out=rT[:], in_=rT[:], func=FX.Sigmoid)

            zT = singles.tile([128, NC, B], f32, tag="zT")
            nc.vector.tensor_add(zT[:], arx[:, 8:16, :], _bc(bsum[:, 8:16], 2, 0, B))
            nc.scalar.activation(out=zT[:], in_=zT[:], func=FX.Sigmoid)

            nT = singles.tile([128, NC, B], f32, tag="nT")
            nc.vector.tensor_add(nT[:], arx[:, 24:32, :], _bc(blob_sb[:, 56:64], 2, 0, B))
            nc.vector.tensor_mul(nT[:], nT[:], rT[:])
            nc.vector.tensor_add(nT[:], nT[:], arx[:, 16:24, :])
            nc.vector.tensor_add(nT[:], nT[:], _bc(blob_sb[:, 32:40], 2, 0, B))
            nc.scalar.activation(out=nT[:], in_=nT[:], func=FX.Tanh)

            h1T = singles.tile([128, NC, B], f32, tag="h1T")
            nc.vector.tensor_mul(h1T[:], zT[:], arx[:, 32:40, :])   # z*h0
            nc.vector.tensor_mul(zT[:], zT[:], nT[:])               # z*n
            nc.vector.tensor_add(h1T[:], h1T[:], nT[:])             # + n
            nc.vector.tensor_sub(h1T[:], h1T[:], zT[:])             # - z*n
            h1q = singles.tile([128, NC, B], f8e4, tag="h1q")
            nc.vector.tensor_scalar_mul(h1q[:], h1T[:], HSCALE)

            # ---- projection (DoubleRow fp8) + online sumexp ---------------
            logits_sb = singles.tile([B, VC], f32, tag="logits_sb")
            s_run = singles.tile([B, 1], f32, tag="s_run")
            nc.vector.memset(s_run, 0.0)

            for g in range(NG):
                col = g * 512
                lg = proj_ps.tile([B, 512], f32, tag="lg")
                for sub in range(2):
                    for k2 in range(4):
                        nc.tensor.matmul(
                            lg[:, sub * 256 : (sub + 1) * 256],
                            h1q[:, 2 * k2 : 2 * k2 + 2, :],
                            pw_sb[:, g, k2, sub],
                            start=(k2 == 0), stop=(k2 == 3),
                            perf_mode=DR,
                        )
                nc.vector.tensor_add(
                    logits_sb[:, col : col + 512], lg[:], pbb[:, col : col + 512]
                )
                expb = stats.tile([B, 512], f32, tag="expb")
                csum = stats.tile([B, 1], f32, tag="csum")
                nc.scalar.activation(
                    out=expb[:], in_=logits_sb[:, col : col + 512], func=FX.Exp,
                    scale=OSCALE, accum_out=csum[:, 0:1],
                )
                nc.vector.tensor_add(s_run, s_run, csum)

            # ---- global sumexp via the second XOR-slot exchange -----------
            sstage = singles.tile([128, 1], f32, tag="sstage")
            nc.vector.tensor_copy(sstage[0:B, :], s_run[:])
            s_all = singles.tile([128, NC], f32, tag="s_all")
            nc.vector.tensor_copy(s_all[:, 0:1], sstage[:])
            # these preps keep their real dep on sstage so they enter the
            # SWDGE ring strictly after the gates-exchange trigger
            for k in range(1, NC):
                nc.gpsimd.remote_dma_broadcast(
                    out_ap=s_all[:, k : k + 1],
                    in_ap=sstage[:],
                    remote_sem=ex2, local_sem=loc2,
                    rdests=[(0, k) if j == k else None for j in range(NC)],
                )
            # padding ops: keep the trigger from racing the last prep on the
            # second Q7 core (desc-gen ~0.9us; each pad ~0.3us dispatch)
            gate2 = stats.tile([1, 1], f32, tag="gate2")
            for _ in range(4):
                nc.gpsimd.tensor_copy(gate2, sstage[0:1, 0:1])
            nc.gpsimd.trigger_dma(count=NC - 1)
            # preload Ln + Identity tables while the exchange is in flight
            tl2 = stats.tile([B, 1], f32, tag="tl2")
            nc.scalar.activation(out=tl2, in_=s_run[:], func=FX.Ln)
            nc.scalar.activation(out=tl2, in_=tl2, func=FX.Identity, scale=1.0, bias=0.0)
            gS = singles.tile([B, 1], f32, tag="gS")
            i_red2 = nc.vector.reduce_sum(gS, s_all[0:B, :], axis=AX.X)
            ngS = singles.tile([B, 1], f32, tag="ngS")
            nc.scalar.activation(out=ngS, in_=gS[:], func=FX.Ln)
            nc.vector.tensor_scalar_mul(ngS, ngS, -1.0)    # -lse

            # ---- logp = logits*OSCALE - lse, write out (split engines,
            # bf16 output halves writeback bytes; host upcasts) ------------
            lp16 = singles.tile([B, VC], bf16, tag="lp16")
            nc.vector.tensor_scalar(
                out=lp16[:, 0:TS_V], in0=logits_sb[:, 0:TS_V],
                scalar1=OSCALE, scalar2=ngS[:, 0:1], op0=ALU.mult, op1=ALU.add,
            )
            nc.scalar.activation(
                out=lp16[:, TS_V:VC], in_=logits_sb[:, TS_V:VC],
                func=FX.Identity, scale=OSCALE, bias=ngS[:, 0:1],
            )
            nc.sync.dma_start(out=logp[:, 0:TS_V], in_=lp16[:, 0:TS_V])
            nc.sync.dma_start(out=logp[:, TS_V:VC], in_=lp16[:, TS_V:VC])

    # attach the cross-core arrival waits post-scheduling: 7 remote senders
    # x 2 sem increments each (the single-core scheduling sim would deadlock
    # on these since it can't model remote increments)
    i_red1.wait_op(ex1, 14, "sem-ge", check=False)
    i_red2.wait_op(ex2, 14, "sem-ge", check=False)

    nc.compile()
    return nc


def kernel(input, hidden, emb, bridge_w, bridge_b, w_ih, w_hh, b_ih, b_hh,
           proj_w, proj_b):
    global _NC_CACHE, LAST_RESULT
    if _NC_CACHE is None:
        _NC_CACHE = _build()
    nc = _NC_CACHE

    input = np.asarray(input)
    hidden = np.asarray(hidden, dtype=np.float32)
    emb = np.asarray(emb, dtype=np.float32)
    bridge_w = np.asarray(bridge_w, dtype=np.float32)
    bridge_b = np.asarray(bridge_b, dtype=np.float32)
    w_ih = np.asarray(w_ih, dtype=np.float32)
    w_hh = np.asarray(w_hh, dtype=np.float32)
    b_ih = np.asarray(b_ih, dtype=np.float32)
    b_hh = np.asarray(b_hh, dtype=np.float32)
    proj_w = np.asarray(proj_w, dtype=np.float32)
    proj_b = np.asarray(proj_b, dtype=np.float32)

    x0 = emb[input[:, 0].astype(np.int64)]          # [B, H]
    x0T = np.ascontiguousarray(x0.T)                # [H, B] f32 (relu on device)
    hid_t = np.ascontiguousarray(hidden.transpose(1, 0, 2)).astype(NP_BF16)

    biT = np.ascontiguousarray(b_ih.reshape(NT, 128).T)   # [128, 24]
    bhT = np.ascontiguousarray(b_hh.reshape(NT, 128).T)

    in_maps = []
    for c in range(NC):
        hs = slice(c * HC, (c + 1) * HC)
        lo, hi = c * VC, min((c + 1) * VC, V)
        pw_blk = proj_w[lo:hi]
        pb_blk = proj_b[lo:hi]
        if hi - lo < VC:
            pad = VC - (hi - lo)
            pw_blk = np.concatenate([pw_blk, np.zeros((pad, H), np.float32)], axis=0)
            pb_blk = np.concatenate([pb_blk, np.full((pad,), NEG, np.float32)])
        # DoubleRow layout: h = kc2*256 + j*128 + p ; v = g*512 + sub*256 + cc
        pwT = np.ascontiguousarray(pw_blk.T) * WSCALE          # [H, VC]
        pw_i = pwT.reshape(4, 2, 128, NG, 2, 256)              # [kc2,j,p,g,sub,c]
        pw_i = pw_i.transpose(2, 3, 0, 4, 1, 5)                # [p,g,kc2,sub,j,c]
        pw_i = np.ascontiguousarray(pw_i).reshape(128, NG * 8 * 256 * 2 // 512 * 512)

        blob = np.zeros((128, BLOB_C), np.float32)
        blob[:, 0:16] = x0T[hs]
        blob[:, 16:40] = biT
        blob[:, 40:64] = bhT
        blob[:, 64] = bridge_w[0]
        blob[:, 65] = bridge_b[0]
        blob[:, 66 + c] = 1.0                                  # mask one-hot

        ebuf = np.concatenate([
            hid_t[:, :, hs].reshape(L, B * HC).astype(NP_BF16),
            np.ascontiguousarray(w_ih[:, hs].T).astype(NP_BF16),
            np.ascontiguousarray(w_hh[:, hs].T).astype(NP_BF16),
        ], axis=1)
        in_maps.append({
            "blob": blob,
            "hid": np.ascontiguousarray(ebuf),
            "pw8": pw_i.astype(NP_F8E4),
            "pb": np.ascontiguousarray((pb_blk * WSCALE).reshape(1, VC)),
        })

    res = run_bass_kernel_spmd(nc, in_maps, list(range(NC)))
    LAST_RESULT = res

    logp_full = np.concatenate(
        [res.results[c]["logp"].astype(np.float32) for c in range(NC)], axis=1
    )
    logp_full = np.ascontiguousarray(logp_full[:, :V])
    return np.broadcast_to(logp_full[:, None, :], (B, L - 1, V))


# revision 51
# speedup vs baseline: 51.7072x; 1.0160x over previous
"""GRU-decoder kernel for 8 Trainium2 NeuronCores.

Math (all 127 output steps are identical — see the reference):
    x0   = relu(emb[input[:,0]])                       [B,H]
    h0   = einsum('blh,l->bh', hidden, bridge_w) + bb  [B,H]
    gi   = x0 @ w_ih.T + b_ih ; gh = h0 @ w_hh.T + b_hh
    r,z  = sigmoid(...) ; n = tanh(in + r*hn)
    h1   = (1-z)*n + z*h0
    logp = log_softmax(h1 @ proj_w.T + proj_b)         [B,V]
    out  = broadcast(logp, [B, L-1, V])

Sharding: vocab-parallel projection (each core owns V/8 rows of proj_w,
stored fp8e4 scaled x512, DoubleRow matmuls) plus h-sharded GRU (each
core owns a 128-wide slice of the hidden dim, computes partial gate
pre-activations, one slim bf16 AllReduce combines them).  True logits
are bounded (|logit| < ~6) so softmax runs without max subtraction; the
only global stat is sumexp, combined with a tiny AllGather + ones-matmul
reduction.  The [B,V] result is gathered on host and broadcast (a
zero-copy view) over the L-1 steps.

Scheduling notes:
  - sync HWDGE ring: 13 x 512KB fp8 weight-stream DMAs, then the
    post-projection stats/output DMAs (ring is idle by then).
  - scalar HWDGE ring: blob of small tensors (one DMA), hidden, GRU
    weights, collective bounce buffers.
  - activation tables (Sigmoid/Tanh/Exp/Ln) are preloaded with dummy
    ops during the initial DMA wait so no table load sits on the
    critical path.
  - a dummy 32B AllGather issued at t=0 warms up the collectives
    firmware before the real AllReduce.
"""

import numpy as np
import ml_dtypes

import concourse.bass as bass
import concourse.tile as tile
from concourse import bacc, mybir
from concourse.bass_utils import run_bass_kernel_spmd

B, L, H, V = 16, 128, 1024, 50257
NC = 8
VC = 6656                # per-core vocab shard (13*512); 8*VC = 53248 >= V
HC = H // NC             # per-core hidden-dim shard (128)
G3 = 3 * H               # gate rows (r,z,n)
NT = G3 // 128           # 24 j-tiles of 128
NG = VC // 512           # 13 projection column groups of 512
NEG = -1.0e30

WSCALE = 512.0           # proj_w scaled by 2^9 so fp8e4 values are normal
HSCALE = 16.0            # h1 scaled by 2^4 before fp8e4 cast
OSCALE = 1.0 / (WSCALE * HSCALE)   # logits de-scale: 2^-13

f32 = mybir.dt.float32
bf16 = mybir.dt.bfloat16
f8e4 = mybir.dt.float8e4
FX = mybir.ActivationFunctionType
AX = mybir.AxisListType
ALU = mybir.AluOpType
DR = mybir.MatmulPerfMode.DoubleRow

NP_F8E4 = ml_dtypes.float8_e4m3
NP_BF16 = ml_dtypes.bfloat16

# blob column layout (f32, [128, 74]):
#   0:16 x0T | 16:40 biT | 40:64 bhT | 64 bw | 65 bb | 66:74 msk
BLOB_C = 74

# AllReduce payload tiles (each [128, tile, B] bf16):
#   0:16 gi+gh partials for r,z | 16:24 in_ | 24:32 hn | 32:40 h0 masked
AR_T = 40

# tail split: vector handles the first TS_V cols, scalar engine the rest
TS_V = 8 * 512

LAST_RESULT = None  # test harness reads profiling info from here
_NC_CACHE = None


def _bc(ap, insert_at, step, count):
    """Insert a broadcast/strided dim into an AP at position insert_at."""
    new = list(ap.ap)
    new.insert(insert_at, [step, count])
    return bass.AP(tensor=ap.tensor, offset=ap.offset, ap=new)


def _build():
    nc = bacc.Bacc("TRN2", target_bir_lowering=False, debug=False, num_devices=NC)

    blob = nc.dram_tensor("blob", [128, BLOB_C], f32, kind="ExternalInput").ap()
    # hid carries [hid | wihT | whhT] packed as one [128, 8192] bf16 tensor
    hid = nc.dram_tensor("hid", [L, 2048 + 2 * G3], bf16, kind="ExternalInput").ap()
    pw8 = nc.dram_tensor("pw8", [128, NG * 8 * 512], f8e4, kind="ExternalInput").ap()
    pb = nc.dram_tensor("pb", [1, VC], f32, kind="ExternalInput").ap()
    logp = nc.dram_tensor("logp", [B, VC], bf16, kind="ExternalOutput").ap()

    ex1 = nc.alloc_semaphore("ex1")   # gates exchange: peers' payload arrived
    loc1 = nc.alloc_semaphore("loc1")
    ex2 = nc.alloc_semaphore("ex2")   # stats exchange
    loc2 = nc.alloc_semaphore("loc2")
    prep1 = nc.alloc_semaphore("prep1")  # descriptor-write completion gates
    prep2 = nc.alloc_semaphore("prep2")

    with tile.TileContext(nc) as tc:
        with (
            tc.tile_pool(name="singles", bufs=1) as singles,
            tc.tile_pool(name="gru_ps", bufs=1, space="PSUM") as gru_ps,
            tc.tile_pool(name="proj_ps", bufs=3, space="PSUM") as proj_ps,
            tc.tile_pool(name="gs_ps", bufs=1, space="PSUM") as gs_pool,
            tc.tile_pool(name="stats", bufs=4) as stats,
            tc.tile_pool(name="dram", bufs=1, space="DRAM") as dram,
        ):
            # ---- token ncfw collective: a NEFF containing a collective is
            # group-launched by the runtime (~20us core stagger); without
            # one the 8 cores dispatch serially ~1.1ms apart.  Gathers
            # garbage, output unused, never on the critical path.
            wcc_in = dram.tile([1, 8], f32, tag="wcc_in")
            wcc_out = dram.tile([NC, 8], f32, tag="wcc_out")
            nc.gpsimd.collective_compute(
                "AllGather", ALU.bypass,
                replica_groups=[list(range(NC))],
                ins=[wcc_in.opt()], outs=[wcc_out.opt()],
            )

            # ---- critical loads first on the sync HWDGE ring --------------
            # one [128, 8192] bf16 transfer carries hid | wihT | whhT; views
            # are hand-built APs into it (one DMA fixed cost instead of 3)
            ebuf = singles.tile([128, 2048 + 2 * G3], bf16, tag="ebuf")
            nc.sync.dma_start(out=ebuf, in_=hid)
            hid_sb = ebuf[:, 0:2048].rearrange("p (b h) -> p b h", b=B)
            wih_sb = ebuf[:, 2048 : 2048 + G3]
            whh_sb = ebuf[:, 2048 + G3 : 2048 + 2 * G3]

            # small/noncritical on the scalar ring
            blob_sb = singles.tile([128, BLOB_C], f32, tag="blob_sb")
            nc.scalar.dma_start(out=blob_sb, in_=blob)
            pbb = singles.tile([B, VC], f32, tag="pbb")
            nc.scalar.dma_start(out=pbb, in_=_bc(pb[0], 0, 0, B))

            # ---- fp8 weight stream (sync ring, after the critical loads;
            # 13 moderate transfers keep SDMA packet turns short) ----------
            # host layout per partition: [g(13), kc2(4), sub(2), j(2), c(256)]
            pw_sb = singles.tile([128, NG, 4, 2, 2, 256], f8e4, tag="pw_sb")
            pw_view = pw8.rearrange("p (g x) -> p g x", g=NG)
            for g in range(NG):
                nc.sync.dma_start(out=pw_sb[:, g], in_=pw_view[:, g])

            # ---- unpack blob ----------------------------------------------
            x0T_sb = singles.tile([HC, B], bf16, tag="x0T_sb")
            nc.scalar.activation(out=x0T_sb[:], in_=blob_sb[:, 0:16], func=FX.Relu)
            bw_sb = singles.tile([L, 1], bf16, tag="bw_sb")
            nc.vector.tensor_copy(bw_sb[:], blob_sb[:, 64:65])
            msk_sb = singles.tile([128, NC], bf16, tag="msk_sb")
            nc.vector.tensor_copy(msk_sb[:], blob_sb[:, 66:74])
            bsum = singles.tile([128, 16], f32, tag="bsum")
            nc.vector.tensor_add(bsum, blob_sb[:, 16:32], blob_sb[:, 40:56])

            # ---- bridge: h0T_c[h,b] = sum_l hidden[b,l,h]*w[l] ------------
            h0T_ps = gru_ps.tile([HC, B], f32, tag="h0T_ps")
            for b in range(B):
                nc.tensor.matmul(
                    h0T_ps[:, b : b + 1], hid_sb[:, b, :], bw_sb[:],
                    start=True, stop=True,
                )
            h0T_sb = singles.tile([HC, B], bf16, tag="h0T_sb")
            nc.vector.tensor_scalar_add(h0T_sb[:], h0T_ps[:], blob_sb[:, 65:66])

            # ---- partial gate pre-activations (T layout) ------------------
            # r,z gates: gi+gh accumulated in one PSUM group; n gate split.
            grz_ps = gru_ps.tile([128, 16, B], f32, tag="grz_ps")
            gin_ps = gru_ps.tile([128, 8, B], f32, tag="gin_ps")
            ghn_ps = gru_ps.tile([128, 8, B], f32, tag="ghn_ps")
            for t in range(16):
                nc.tensor.matmul(
                    grz_ps[:, t, :], wih_sb[:, t * 128 : (t + 1) * 128], x0T_sb[:],
                    start=True, stop=False,
                )
                nc.tensor.matmul(
                    grz_ps[:, t, :], whh_sb[:, t * 128 : (t + 1) * 128], h0T_sb[:],
                    start=False, stop=True,
                )
            for t in range(16, NT):
                nc.tensor.matmul(
                    gin_ps[:, t - 16, :], wih_sb[:, t * 128 : (t + 1) * 128], x0T_sb[:],
                    start=True, stop=True,
                )
                nc.tensor.matmul(
                    ghn_ps[:, t - 16, :], whh_sb[:, t * 128 : (t + 1) * 128], h0T_sb[:],
                    start=True, stop=True,
                )

            # ---- pack slim exchange payload [128, 40, 16] bf16 ------------
            arbuf = singles.tile([128, AR_T, B], bf16, tag="arbuf")
            nc.vector.tensor_copy(arbuf[:, 0:16, :], grz_ps[:])
            nc.vector.tensor_copy(arbuf[:, 16:24, :], gin_ps[:])
            nc.vector.tensor_copy(arbuf[:, 24:32, :], ghn_ps[:])
            h0_bcast = _bc(h0T_sb[:], 1, 0, NC)          # [128, 8, 16]
            msk_bcast = _bc(msk_sb[:], 2, 0, B)          # [128, 8, 16]
            nc.vector.tensor_mul(arbuf[:, 32:40, :], h0_bcast, msk_bcast)

            # ---- all-to-all exchange of partials (no ncfw, XOR slots) -----
            # broadcast #k sends my payload to core (me XOR k), into static
            # slot k.  Receiver d's slot k holds sender d^k — a permutation,
            # and the summed reduction is order-invariant.  Slot 0 (self) is
            # filled by a local copy.  Descriptor preps encode addresses
            # only, so they are dep-decoupled (decoy tracking offset) and
            # run early; the single trigger is gated on the packed payload
            # through a tiny gpsimd op that genuinely reads arbuf.
            arx_all = singles.tile([128, NC, AR_T * B], bf16, tag="arx_all")
            ab = arbuf[:]
            ab_flat = bass.AP(tensor=ab.tensor, offset=ab.offset,
                              ap=[ab.ap[0], [1, AR_T * B]])
            preps1 = []
            for k in range(1, NC):
                preps1.append(nc.gpsimd.remote_dma_broadcast(
                    out_ap=arx_all[:, k, :],
                    in_ap=ab_flat,
                    remote_sem=ex1, local_sem=loc1,
                    rdests=[(0, k) if j == k else None for j in range(NC)],
                ))
            nc.vector.tensor_copy(arx_all[:, 0, :], ab_flat)
            trig1 = nc.gpsimd.trigger_dma(count=NC - 1)
            # hard sync deps: the auto-attached no-sync deps carry no
            # semaphores, so the second Q7 core can run the trigger before
            # the last descriptor prep finishes (observed on HW)
            _d1 = bass.InstructionNameOrderedSet()
            for p in preps1:
                _d1.add(p.ins.name)
            trig1.ins.add_sync_dependencies_from(_d1)
            # preload sigmoid/tanh tables while the exchange is in flight
            tl = stats.tile([128, 1], f32, tag="tl")
            nc.scalar.activation(out=tl, in_=arbuf[:, 0:1, 0:1], func=FX.Tanh)
            nc.scalar.activation(out=tl, in_=tl, func=FX.Sigmoid)
            # the ex1>=14 wait (7 remote senders x 2 increments) is attached
            # to i_red1 AFTER tile scheduling — the single-core scheduling
            # sim can't see remote increments and would declare deadlock
            arxv = arx_all[:].rearrange("p c (t b) -> p c t b", t=AR_T)
            arx = singles.tile([128, AR_T, B], f32, tag="arx")
            i_red1 = nc.vector.tensor_add(arx[:], arxv[:, 0], arxv[:, 1])
            for k in range(2, NC):
                nc.vector.tensor_add(arx[:], arx[:], arxv[:, k])

            # ---- gates (full width, every core redundantly) ---------------
            rT = singles.tile([128, NC, B], f32, tag="rT")
            nc.vector.tensor_add(rT[:], arx[:, 0:8, :], _bc(bsum[:, 0:8], 2, 0, B))
            nc.scalar.activation(out=rT[:], in_=rT[:], func=FX.Sigmoid)

            zT = singles.tile([128, NC, B], f32, tag="zT")
            nc.vector.tensor_add(zT[:], arx[:, 8:16, :], _bc(bsum[:, 8:16], 2, 0, B))
            nc.scalar.activation(out=zT[:], in_=zT[:], func=FX.Sigmoid)

            nT = singles.tile([128, NC, B], f32, tag="nT")
            nc.vector.tensor_add(nT[:], arx[:, 24:32, :], _bc(blob_sb[:, 56:64], 2, 0, B))
            nc.vector.tensor_mul(nT[:], nT[:], rT[:])
            nc.vector.tensor_add(nT[:], nT[:], arx[:, 16:24, :])
            nc.vector.tensor_add(nT[:], nT[:], _bc(blob_sb[:, 32:40], 2, 0, B))
            nc.scalar.activation(out=nT[:], in_=nT[:], func=FX.Tanh)

            h1T = singles.tile([128, NC, B], f32, tag="h1T")
            nc.vector.tensor_mul(h1T[:], zT[:], arx[:, 32:40, :])   # z*h0
            nc.vector.tensor_mul(zT[:], zT[:], nT[:])               # z*n
            nc.vector.tensor_add(h1T[:], h1T[:], nT[:])             # + n
            nc.vector.tensor_sub(h1T[:], h1T[:], zT[:])             # - z*n
            h1q = singles.tile([128, NC, B], f8e4, tag="h1q")
            nc.vector.tensor_scalar_mul(h1q[:], h1T[:], HSCALE)

            # ---- projection (DoubleRow fp8) + online sumexp ---------------
            logits_sb = singles.tile([B, VC], f32, tag="logits_sb")
            s_run = singles.tile([B, 1], f32, tag="s_run")
            nc.vector.memset(s_run, 0.0)

            for g in range(NG):
                col = g * 512
                lg = proj_ps.tile([B, 512], f32, tag="lg")
                for sub in range(2):
                    for k2 in range(4):
                        nc.tensor.matmul(
                            lg[:, sub * 256 : (sub + 1) * 256],
                            h1q[:, 2 * k2 : 2 * k2 + 2, :],
                            pw_sb[:, g, k2, sub],
                            start=(k2 == 0), stop=(k2 == 3),
                            perf_mode=DR,
                        )
                nc.vector.tensor_add(
                    logits_sb[:, col : col + 512], lg[:], pbb[:, col : col + 512]
                )
                expb = stats.tile([B, 512], f32, tag="expb")
                csum = stats.tile([B, 1], f32, tag="csum")
                nc.scalar.activation(
                    out=expb[:], in_=logits_sb[:, col : col + 512], func=FX.Exp,
                    scale=OSCALE, accum_out=csum[:, 0:1],
                )
                nc.vector.tensor_add(s_run, s_run, csum)

            # ---- global sumexp via the second XOR-slot exchange -----------
            sstage = singles.tile([128, 1], f32, tag="sstage")
            nc.vector.tensor_copy(sstage[0:B, :], s_run[:])
            s_all = singles.tile([128, NC], f32, tag="s_all")
            nc.vector.tensor_copy(s_all[:, 0:1], sstage[:])
            # these preps keep their real dep on sstage so they enter the
            # SWDGE ring strictly after the gates-exchange trigger
            preps2 = []
            for k in range(1, NC):
                preps2.append(nc.gpsimd.remote_dma_broadcast(
                    out_ap=s_all[:, k : k + 1],
                    in_ap=sstage[:],
                    remote_sem=ex2, local_sem=loc2,
                    rdests=[(0, k) if j == k else None for j in range(NC)],
                ))
            trig2 = nc.gpsimd.trigger_dma(count=NC - 1)
            _d2 = bass.InstructionNameOrderedSet()
            for p in preps2:
                _d2.add(p.ins.name)
            trig2.ins.add_sync_dependencies_from(_d2)
            # preload Ln + Identity tables while the exchange is in flight
            tl2 = stats.tile([B, 1], f32, tag="tl2")
            nc.scalar.activation(out=tl2, in_=s_run[:], func=FX.Ln)
            nc.scalar.activation(out=tl2, in_=tl2, func=FX.Identity, scale=1.0, bias=0.0)
            gS = singles.tile([B, 1], f32, tag="gS")
            i_red2 = nc.vector.reduce_sum(gS, s_all[0:B, :], axis=AX.X)
            ngS = singles.tile([B, 1], f32, tag="ngS")
            nc.scalar.activation(out=ngS, in_=gS[:], func=FX.Ln)
            nc.vector.tensor_scalar_mul(ngS, ngS, -1.0)    # -lse

            # ---- logp = logits*OSCALE - lse, write out (split engines,
            # bf16 output halves writeback bytes; host upcasts) ------------
            lp16 = singles.tile([B, VC], bf16, tag="lp16")
            nc.vector.tensor_scalar(
                out=lp16[:, 0:TS_V], in0=logits_sb[:, 0:TS_V],
                scalar1=OSCALE, scalar2=ngS[:, 0:1], op0=ALU.mult, op1=ALU.add,
            )
            nc.scalar.activation(
                out=lp16[:, TS_V:VC], in_=logits_sb[:, TS_V:VC],
                func=FX.Identity, scale=OSCALE, bias=ngS[:, 0:1],
            )
            nc.sync.dma_start(out=logp[:, 0:TS_V], in_=lp16[:, 0:TS_V])
            nc.sync.dma_start(out=logp[:, TS_V:VC], in_=lp16[:, TS_V:VC])

    # attach the cross-core arrival waits post-scheduling: 7 remote senders
    # x 2 sem increments each (the single-core scheduling sim would deadlock
    # on these since it can't model remote increments)
    i_red1.wait_op(ex1, 14, "sem-ge", check=False)
    i_red2.wait_op(ex2, 14, "sem-ge", check=False)

    nc.compile()
    return nc


def kernel(input, hidden, emb, bridge_w, bridge_b, w_ih, w_hh, b_ih, b_hh,
           proj_w, proj_b):
    global _NC_CACHE, LAST_RESULT
    if _NC_CACHE is None:
        _NC_CACHE = _build()
    nc = _NC_CACHE

    input = np.asarray(input)
    hidden = np.asarray(hidden, dtype=np.float32)
    emb = np.asarray(emb, dtype=np.float32)
    bridge_w = np.asarray(bridge_w, dtype=np.float32)
    bridge_b = np.asarray(bridge_b, dtype=np.float32)
    w_ih = np.asarray(w_ih, dtype=np.float32)
    w_hh = np.asarray(w_hh, dtype=np.float32)
    b_ih = np.asarray(b_ih, dtype=np.float32)
    b_hh = np.asarray(b_hh, dtype=np.float32)
    proj_w = np.asarray(proj_w, dtype=np.float32)
    proj_b = np.asarray(proj_b, dtype=np.float32)

    x0 = emb[input[:, 0].astype(np.int64)]          # [B, H]
    x0T = np.ascontiguousarray(x0.T)                # [H, B] f32 (relu on device)
    hid_t = np.ascontiguousarray(hidden.transpose(1, 0, 2)).astype(NP_BF16)

    biT = np.ascontiguousarray(b_ih.reshape(NT, 128).T)   # [128, 24]
    bhT = np.ascontiguousarray(b_hh.reshape(NT, 128).T)

    in_maps = []
    for c in range(NC):
        hs = slice(c * HC, (c + 1) * HC)
        lo, hi = c * VC, min((c + 1) * VC, V)
        pw_blk = proj_w[lo:hi]
        pb_blk = proj_b[lo:hi]
        if hi - lo < VC:
            pad = VC - (hi - lo)
            pw_blk = np.concatenate([pw_blk, np.zeros((pad, H), np.float32)], axis=0)
            pb_blk = np.concatenate([pb_blk, np.full((pad,), NEG, np.float32)])
        # DoubleRow layout: h = kc2*256 + j*128 + p ; v = g*512 + sub*256 + cc
        pwT = np.ascontiguousarray(pw_blk.T) * WSCALE          # [H, VC]
        pw_i = pwT.reshape(4, 2, 128, NG, 2, 256)              # [kc2,j,p,g,sub,c]
        pw_i = pw_i.transpose(2, 3, 0, 4, 1, 5)                # [p,g,kc2,sub,j,c]
        pw_i = np.ascontiguousarray(pw_i).reshape(128, NG * 8 * 256 * 2 // 512 * 512)

        blob = np.zeros((128, BLOB_C), np.float32)
        blob[:, 0:16] = x0T[hs]
        blob[:, 16:40] = biT
        blob[:, 40:64] = bhT
        blob[:, 64] = bridge_w[0]
        blob[:, 65] = bridge_b[0]
        blob[:, 66 + c] = 1.0                                  # mask one-hot

        ebuf = np.concatenate([
            hid_t[:, :, hs].reshape(L, B * HC).astype(NP_BF16),
            np.ascontiguousarray(w_ih[:, hs].T).astype(NP_BF16),
            np.ascontiguousarray(w_hh[:, hs].T).astype(NP_BF16),
        ], axis=1)
        in_maps.append({
            "blob": blob,
            "hid": np.ascontiguousarray(ebuf),
            "pw8": pw_i.astype(NP_F8E4),
            "pb": np.ascontiguousarray((pb_blk * WSCALE).reshape(1, VC)),
        })

    res = run_bass_kernel_spmd(nc, in_maps, list(range(NC)))
    LAST_RESULT = res

    logp_full = np.concatenate(
        [res.results[c]["logp"].astype(np.float32) for c in range(NC)], axis=1
    )
    logp_full = np.ascontiguousarray(logp_full[:, :V])
    return np.broadcast_to(logp_full[:, None, :], (B, L - 1, V))


# revision 52
# speedup vs baseline: 66.4287x; 1.2847x over previous
"""GRU-decoder kernel for 8 Trainium2 NeuronCores.

Math (all 127 output steps are identical — see the reference):
    x0   = relu(emb[input[:,0]])                       [B,H]
    h0   = einsum('blh,l->bh', hidden, bridge_w) + bb  [B,H]
    gi   = x0 @ w_ih.T + b_ih ; gh = h0 @ w_hh.T + b_hh
    r,z  = sigmoid(...) ; n = tanh(in + r*hn)
    h1   = (1-z)*n + z*h0
    logp = log_softmax(h1 @ proj_w.T + proj_b)         [B,V]
    out  = broadcast(logp, [B, L-1, V])

Sharding: vocab-parallel projection (each core owns V/8 rows of proj_w,
stored fp8e4 scaled x512, DoubleRow matmuls) plus h-sharded GRU (each
core owns a 128-wide slice of the hidden dim, computes partial gate
pre-activations, one slim bf16 AllReduce combines them).  True logits
are bounded (|logit| < ~6) so softmax runs without max subtraction; the
only global stat is sumexp, combined with a tiny AllGather + ones-matmul
reduction.  The [B,V] result is gathered on host and broadcast (a
zero-copy view) over the L-1 steps.

Scheduling notes:
  - sync HWDGE ring: 13 x 512KB fp8 weight-stream DMAs, then the
    post-projection stats/output DMAs (ring is idle by then).
  - scalar HWDGE ring: blob of small tensors (one DMA), hidden, GRU
    weights, collective bounce buffers.
  - activation tables (Sigmoid/Tanh/Exp/Ln) are preloaded with dummy
    ops during the initial DMA wait so no table load sits on the
    critical path.
  - a dummy 32B AllGather issued at t=0 warms up the collectives
    firmware before the real AllReduce.
"""

import numpy as np
import ml_dtypes

import concourse.bass as bass
import concourse.tile as tile
from concourse import bacc, mybir
from concourse.bass_utils import run_bass_kernel_spmd

B, L, H, V = 16, 128, 1024, 50257
NC = 8
VC = 6656                # per-core vocab shard (13*512); 8*VC = 53248 >= V
HC = H // NC             # per-core hidden-dim shard (128)
G3 = 3 * H               # gate rows (r,z,n)
NT = G3 // 128           # 24 j-tiles of 128
NG = VC // 512           # 13 projection column groups of 512
NEG = -1.0e30

WSCALE = 512.0           # proj_w scaled by 2^9 so fp8e4 values are normal
HSCALE = 16.0            # h1 scaled by 2^4 before fp8e4 cast
OSCALE = 1.0 / (WSCALE * HSCALE)   # logits de-scale: 2^-13

f32 = mybir.dt.float32
bf16 = mybir.dt.bfloat16
f8e4 = mybir.dt.float8e4
FX = mybir.ActivationFunctionType
AX = mybir.AxisListType
ALU = mybir.AluOpType
DR = mybir.MatmulPerfMode.DoubleRow

NP_F8E4 = ml_dtypes.float8_e4m3
NP_BF16 = ml_dtypes.bfloat16

# blob column layout (f32, [128, 74]):
#   0:16 x0T | 16:40 biT | 40:64 bhT | 64 bw | 65 bb | 66:74 msk
BLOB_C = 74

# AllReduce payload tiles (each [128, tile, B] bf16):
#   0:16 gi+gh partials for r,z | 16:24 in_ | 24:32 hn | 32:40 h0 masked
AR_T = 40

# tail split: vector handles the first TS_V cols, scalar engine the rest
TS_V = 8 * 512

LAST_RESULT = None  # test harness reads profiling info from here
_NC_CACHE = None


def _bc(ap, insert_at, step, count):
    """Insert a broadcast/strided dim into an AP at position insert_at."""
    new = list(ap.ap)
    new.insert(insert_at, [step, count])
    return bass.AP(tensor=ap.tensor, offset=ap.offset, ap=new)


def _build():
    nc = bacc.Bacc("TRN2", target_bir_lowering=False, debug=False, num_devices=NC)

    blob = nc.dram_tensor("blob", [128, BLOB_C], f32, kind="ExternalInput").ap()
    # hid carries [hid | wihT | whhT] packed as one [128, 8192] bf16 tensor
    hid = nc.dram_tensor("hid", [L, 2048 + 2 * G3], bf16, kind="ExternalInput").ap()
    pw8 = nc.dram_tensor("pw8", [128, NG * 8 * 512], f8e4, kind="ExternalInput").ap()
    pb = nc.dram_tensor("pb", [1, VC], f32, kind="ExternalInput").ap()
    logp = nc.dram_tensor("logp", [B, VC], bf16, kind="ExternalOutput").ap()

    with tile.TileContext(nc) as tc:
        with (
            tc.tile_pool(name="singles", bufs=1) as singles,
            tc.tile_pool(name="gru_ps", bufs=1, space="PSUM") as gru_ps,
            tc.tile_pool(name="proj_ps", bufs=3, space="PSUM") as proj_ps,
            tc.tile_pool(name="gs_ps", bufs=1, space="PSUM") as gs_pool,
            tc.tile_pool(name="stats", bufs=4) as stats,
            tc.tile_pool(name="dram", bufs=1, space="DRAM") as dram,
        ):
            # ---- dummy collective: wake the ncfw firmware early -----------
            # (gathers garbage DRAM; output unused — only the barrier and
            # firmware warm-up matter)
            wcc_in = dram.tile([1, 8], f32, tag="wcc_in")
            wcc_out = dram.tile([NC, 8], f32, tag="wcc_out")
            nc.gpsimd.collective_compute(
                "AllGather", ALU.bypass,
                replica_groups=[list(range(NC))],
                ins=[wcc_in.opt()], outs=[wcc_out.opt()],
            )

            # ---- critical loads first on the sync HWDGE ring --------------
            # one [128, 8192] bf16 transfer carries hid | wihT | whhT; views
            # are hand-built APs into it (one DMA fixed cost instead of 3)
            ebuf = singles.tile([128, 2048 + 2 * G3], bf16, tag="ebuf")
            nc.sync.dma_start(out=ebuf, in_=hid)
            hid_sb = ebuf[:, 0:2048].rearrange("p (b h) -> p b h", b=B)
            wih_sb = ebuf[:, 2048 : 2048 + G3]
            whh_sb = ebuf[:, 2048 + G3 : 2048 + 2 * G3]

            # small/noncritical on the scalar ring
            blob_sb = singles.tile([128, BLOB_C], f32, tag="blob_sb")
            nc.scalar.dma_start(out=blob_sb, in_=blob)
            pbb = singles.tile([B, VC], f32, tag="pbb")
            nc.scalar.dma_start(out=pbb, in_=_bc(pb[0], 0, 0, B))

            # ---- fp8 weight stream (sync ring, after the critical loads;
            # 13 moderate transfers keep SDMA packet turns short) ----------
            # host layout per partition: [g(13), kc2(4), sub(2), j(2), c(256)]
            pw_sb = singles.tile([128, NG, 4, 2, 2, 256], f8e4, tag="pw_sb")
            pw_view = pw8.rearrange("p (g x) -> p g x", g=NG)
            for g in range(NG):
                nc.sync.dma_start(out=pw_sb[:, g], in_=pw_view[:, g])

            # ---- unpack blob ----------------------------------------------
            x0T_sb = singles.tile([HC, B], bf16, tag="x0T_sb")
            nc.scalar.activation(out=x0T_sb[:], in_=blob_sb[:, 0:16], func=FX.Relu)
            bw_sb = singles.tile([L, 1], bf16, tag="bw_sb")
            nc.vector.tensor_copy(bw_sb[:], blob_sb[:, 64:65])
            msk_sb = singles.tile([128, NC], bf16, tag="msk_sb")
            nc.vector.tensor_copy(msk_sb[:], blob_sb[:, 66:74])
            bsum = singles.tile([128, 16], f32, tag="bsum")
            nc.vector.tensor_add(bsum, blob_sb[:, 16:32], blob_sb[:, 40:56])

            # ---- bridge: h0T_c[h,b] = sum_l hidden[b,l,h]*w[l] ------------
            h0T_ps = gru_ps.tile([HC, B], f32, tag="h0T_ps")
            for b in range(B):
                nc.tensor.matmul(
                    h0T_ps[:, b : b + 1], hid_sb[:, b, :], bw_sb[:],
                    start=True, stop=True,
                )
            h0T_sb = singles.tile([HC, B], bf16, tag="h0T_sb")
            nc.vector.tensor_scalar_add(h0T_sb[:], h0T_ps[:], blob_sb[:, 65:66])

            # ---- partial gate pre-activations (T layout) ------------------
            # r,z gates: gi+gh accumulated in one PSUM group; n gate split.
            grz_ps = gru_ps.tile([128, 16, B], f32, tag="grz_ps")
            gin_ps = gru_ps.tile([128, 8, B], f32, tag="gin_ps")
            ghn_ps = gru_ps.tile([128, 8, B], f32, tag="ghn_ps")
            for t in range(16):
                nc.tensor.matmul(
                    grz_ps[:, t, :], wih_sb[:, t * 128 : (t + 1) * 128], x0T_sb[:],
                    start=True, stop=False,
                )
                nc.tensor.matmul(
                    grz_ps[:, t, :], whh_sb[:, t * 128 : (t + 1) * 128], h0T_sb[:],
                    start=False, stop=True,
                )
            for t in range(16, NT):
                nc.tensor.matmul(
                    gin_ps[:, t - 16, :], wih_sb[:, t * 128 : (t + 1) * 128], x0T_sb[:],
                    start=True, stop=True,
                )
                nc.tensor.matmul(
                    ghn_ps[:, t - 16, :], whh_sb[:, t * 128 : (t + 1) * 128], h0T_sb[:],
                    start=True, stop=True,
                )

            # ---- pack slim AllReduce payload [128, 40, 16] fp8e4 ----------
            arbuf = singles.tile([128, AR_T, B], f8e4, tag="arbuf")
            nc.vector.tensor_copy(arbuf[:, 0:16, :], grz_ps[:])
            nc.vector.tensor_copy(arbuf[:, 16:24, :], gin_ps[:])
            nc.vector.tensor_copy(arbuf[:, 24:32, :], ghn_ps[:])
            h0_bcast = _bc(h0T_sb[:], 1, 0, NC)          # [128, 8, 16]
            msk_bcast = _bc(msk_sb[:], 2, 0, B)          # [128, 8, 16]
            nc.vector.tensor_mul(arbuf[:, 32:40, :], h0_bcast, msk_bcast)

            # SWDGE path: completion tracked on its own sem lanes, not the
            # HWDGE lanes shared with the weight-stream DMAs
            cc_in = dram.tile([128, AR_T * B], f8e4, tag="cc_in")
            cc_out = dram.tile([128, AR_T * B], f8e4, tag="cc_out")
            nc.gpsimd.dma_start(out=cc_in[:], in_=arbuf[:])
            nc.gpsimd.collective_compute(
                "AllReduce", ALU.add,
                replica_groups=[list(range(NC))],
                ins=[cc_in.opt()], outs=[cc_out.opt()],
            )
            # preload sigmoid/tanh tables while the AllReduce is in flight
            tl = stats.tile([128, 1], f32, tag="tl")
            nc.scalar.activation(out=tl, in_=arbuf[:, 0:1, 0:1], func=FX.Tanh)
            nc.scalar.activation(out=tl, in_=tl, func=FX.Sigmoid)
            arx8 = singles.tile([128, AR_T, B], f8e4, tag="arx8")
            nc.scalar.dma_start(out=arx8[:], in_=cc_out[:])
            arx = singles.tile([128, AR_T, B], f32, tag="arx")
            nc.vector.tensor_copy(arx[:], arx8[:])

            # ---- gates (full width, every core redundantly) ---------------
            rT = singles.tile([128, NC, B], f32, tag="rT")
            nc.vector.tensor_add(rT[:], arx[:, 0:8, :], _bc(bsum[:, 0:8], 2, 0, B))
            nc.scalar.activation(out=rT[:], in_=rT[:], func=FX.Sigmoid)

            zT = singles.tile([128, NC, B], f32, tag="zT")
            nc.vector.tensor_add(zT[:], arx[:, 8:16, :], _bc(bsum[:, 8:16], 2, 0, B))
            nc.scalar.activation(out=zT[:], in_=zT[:], func=FX.Sigmoid)

            nT = singles.tile([128, NC, B], f32, tag="nT")
            nc.vector.tensor_add(nT[:], arx[:, 24:32, :], _bc(blob_sb[:, 56:64], 2, 0, B))
            nc.vector.tensor_mul(nT[:], nT[:], rT[:])
            nc.vector.tensor_add(nT[:], nT[:], arx[:, 16:24, :])
            nc.vector.tensor_add(nT[:], nT[:], _bc(blob_sb[:, 32:40], 2, 0, B))
            nc.scalar.activation(out=nT[:], in_=nT[:], func=FX.Tanh)

            h1T = singles.tile([128, NC, B], f32, tag="h1T")
            nc.vector.tensor_mul(h1T[:], zT[:], arx[:, 32:40, :])   # z*h0
            nc.vector.tensor_mul(zT[:], zT[:], nT[:])               # z*n
            nc.vector.tensor_add(h1T[:], h1T[:], nT[:])             # + n
            nc.vector.tensor_sub(h1T[:], h1T[:], zT[:])             # - z*n
            h1q = singles.tile([128, NC, B], f8e4, tag="h1q")
            nc.vector.tensor_scalar_mul(h1q[:], h1T[:], HSCALE)

            # ---- projection (DoubleRow fp8) + online sumexp ---------------
            logits_sb = singles.tile([B, VC], f32, tag="logits_sb")
            s_run = singles.tile([B, 1], f32, tag="s_run")
            nc.vector.memset(s_run, 0.0)

            for g in range(NG):
                col = g * 512
                lg = proj_ps.tile([B, 512], f32, tag="lg")
                for sub in range(2):
                    for k2 in range(4):
                        nc.tensor.matmul(
                            lg[:, sub * 256 : (sub + 1) * 256],
                            h1q[:, 2 * k2 : 2 * k2 + 2, :],
                            pw_sb[:, g, k2, sub],
                            start=(k2 == 0), stop=(k2 == 3),
                            perf_mode=DR,
                        )
                nc.vector.tensor_add(
                    logits_sb[:, col : col + 512], lg[:], pbb[:, col : col + 512]
                )
                expb = stats.tile([B, 512], f32, tag="expb")
                csum = stats.tile([B, 1], f32, tag="csum")
                nc.scalar.activation(
                    out=expb[:], in_=logits_sb[:, col : col + 512], func=FX.Exp,
                    scale=OSCALE, accum_out=csum[:, 0:1],
                )
                nc.vector.tensor_add(s_run, s_run, csum)

            # preload Ln + Identity tables while the AllGather is in flight
            tl2 = stats.tile([B, 1], f32, tag="tl2")
            nc.scalar.activation(out=tl2, in_=s_run[:], func=FX.Ln)
            nc.scalar.activation(out=tl2, in_=tl2, func=FX.Identity, scale=1.0, bias=0.0)

            # ---- global sumexp (AllGather + ones-matmul reduce) -----------
            std_in = dram.tile([1, B], f32, tag="std_in")
            std_out = dram.tile([NC, B], f32, tag="std_out")
            nc.sync.dma_start(out=std_in[0:1, :], in_=s_run[:])
            nc.gpsimd.collective_compute(
                "AllGather", ALU.bypass,
                replica_groups=[list(range(NC))],
                ins=[std_in.opt()], outs=[std_out.opt()],
            )
            sall = singles.tile([NC, B], f32, tag="sall")
            nc.scalar.dma_start(out=sall, in_=std_out[:])
            ones8 = singles.tile([NC, 1], f32, tag="ones8")
            nc.vector.memset(ones8, 1.0)
            gS_ps = gs_pool.tile([B, 1], f32, tag="gS_ps")
            nc.tensor.matmul(gS_ps[:], sall[:], ones8[:], start=True, stop=True)
            ngS = singles.tile([B, 1], f32, tag="ngS")
            nc.scalar.activation(out=ngS, in_=gS_ps[:], func=FX.Ln)
            nc.vector.tensor_scalar_mul(ngS, ngS, -1.0)    # -lse

            # ---- logp = logits*OSCALE - lse, write out (split engines,
            # bf16 output halves writeback bytes; host upcasts) ------------
            lp16 = singles.tile([B, VC], bf16, tag="lp16")
            nc.vector.tensor_scalar(
                out=lp16[:, 0:TS_V], in0=logits_sb[:, 0:TS_V],
                scalar1=OSCALE, scalar2=ngS[:, 0:1], op0=ALU.mult, op1=ALU.add,
            )
            nc.scalar.activation(
                out=lp16[:, TS_V:VC], in_=logits_sb[:, TS_V:VC],
                func=FX.Identity, scale=OSCALE, bias=ngS[:, 0:1],
            )
            nc.sync.dma_start(out=logp[:, 0:TS_V], in_=lp16[:, 0:TS_V])
            nc.sync.dma_start(out=logp[:, TS_V:VC], in_=lp16[:, TS_V:VC])

    nc.compile()
    return nc


def kernel(input, hidden, emb, bridge_w, bridge_b, w_ih, w_hh, b_ih, b_hh,
           proj_w, proj_b):
    global _NC_CACHE, LAST_RESULT
    if _NC_CACHE is None:
        _NC_CACHE = _build()
    nc = _NC_CACHE

    input = np.asarray(input)
    hidden = np.asarray(hidden, dtype=np.float32)
    emb = np.asarray(emb, dtype=np.float32)
    bridge_w = np.asarray(bridge_w, dtype=np.float32)
    bridge_b = np.asarray(bridge_b, dtype=np.float32)
    w_ih = np.asarray(w_ih, dtype=np.float32)
    w_hh = np.asarray(w_hh, dtype=np.float32)
    b_ih = np.asarray(b_ih, dtype=np.float32)
    b_hh = np.asarray(b_hh, dtype=np.float32)
    proj_w = np.asarray(proj_w, dtype=np.float32)
    proj_b = np.asarray(proj_b, dtype=np.float32)

    x0 = emb[input[:, 0].astype(np.int64)]          # [B, H]
    x0T = np.ascontiguousarray(x0.T)                # [H, B] f32 (relu on device)
    hid_t = np.ascontiguousarray(hidden.transpose(1, 0, 2)).astype(NP_BF16)

    biT = np.ascontiguousarray(b_ih.reshape(NT, 128).T)   # [128, 24]
    bhT = np.ascontiguousarray(b_hh.reshape(NT, 128).T)

    in_maps = []
    for c in range(NC):
        hs = slice(c * HC, (c + 1) * HC)
        lo, hi = c * VC, min((c + 1) * VC, V)
        pw_blk = proj_w[lo:hi]
        pb_blk = proj_b[lo:hi]
        if hi - lo < VC:
            pad = VC - (hi - lo)
            pw_blk = np.concatenate([pw_blk, np.zeros((pad, H), np.float32)], axis=0)
            pb_blk = np.concatenate([pb_blk, np.full((pad,), NEG, np.float32)])
        # DoubleRow layout: h = kc2*256 + j*128 + p ; v = g*512 + sub*256 + cc
        pwT = np.ascontiguousarray(pw_blk.T) * WSCALE          # [H, VC]
        pw_i = pwT.reshape(4, 2, 128, NG, 2, 256)              # [kc2,j,p,g,sub,c]
        pw_i = pw_i.transpose(2, 3, 0, 4, 1, 5)                # [p,g,kc2,sub,j,c]
        pw_i = np.ascontiguousarray(pw_i).reshape(128, NG * 8 * 256 * 2 // 512 * 512)

        blob = np.zeros((128, BLOB_C), np.float32)
        blob[:, 0:16] = x0T[hs]
        blob[:, 16:40] = biT
        blob[:, 40:64] = bhT
        blob[:, 64] = bridge_w[0]
        blob[:, 65] = bridge_b[0]
        blob[:, 66 + c] = 1.0                                  # mask one-hot

        ebuf = np.concatenate([
            hid_t[:, :, hs].reshape(L, B * HC).astype(NP_BF16),
            np.ascontiguousarray(w_ih[:, hs].T).astype(NP_BF16),
            np.ascontiguousarray(w_hh[:, hs].T).astype(NP_BF16),
        ], axis=1)
        in_maps.append({
            "blob": blob,
            "hid": np.ascontiguousarray(ebuf),
            "pw8": pw_i.astype(NP_F8E4),
            "pb": np.ascontiguousarray((pb_blk * WSCALE).reshape(1, VC)),
        })

    res = run_bass_kernel_spmd(nc, in_maps, list(range(NC)))
    LAST_RESULT = res

    logp_full = np.concatenate(
        [res.results[c]["logp"].astype(np.float32) for c in range(NC)], axis=1
    )
    logp_full = np.ascontiguousarray(logp_full[:, :V])
    return np.broadcast_to(logp_full[:, None, :], (B, L - 1, V))


# revision 53
# speedup vs baseline: 68.0586x; 1.0245x over previous
"""GRU-decoder kernel for 8 Trainium2 NeuronCores.

Math (all 127 output steps are identical — see the reference):
    x0   = relu(emb[input[:,0]])                       [B,H]
    h0   = einsum('blh,l->bh', hidden, bridge_w) + bb  [B,H]
    gi   = x0 @ w_ih.T + b_ih ; gh = h0 @ w_hh.T + b_hh
    r,z  = sigmoid(...) ; n = tanh(in + r*hn)
    h1   = (1-z)*n + z*h0
    logp = log_softmax(h1 @ proj_w.T + proj_b)         [B,V]
    out  = broadcast(logp, [B, L-1, V])

Sharding: vocab-parallel projection (each core owns V/8 rows of proj_w,
stored fp8e4 scaled x512, DoubleRow matmuls) plus h-sharded GRU (each
core owns a 128-wide slice of the hidden dim, computes partial gate
pre-activations, one slim bf16 AllReduce combines them).  True logits
are bounded (|logit| < ~6) so softmax runs without max subtraction; the
only global stat is sumexp, combined with a tiny AllGather + ones-matmul
reduction.  The [B,V] result is gathered on host and broadcast (a
zero-copy view) over the L-1 steps.

Scheduling notes:
  - sync HWDGE ring: 13 x 512KB fp8 weight-stream DMAs, then the
    post-projection stats/output DMAs (ring is idle by then).
  - scalar HWDGE ring: blob of small tensors (one DMA), hidden, GRU
    weights, collective bounce buffers.
  - activation tables (Sigmoid/Tanh/Exp/Ln) are preloaded with dummy
    ops during the initial DMA wait so no table load sits on the
    critical path.
  - a dummy 32B AllGather issued at t=0 warms up the collectives
    firmware before the real AllReduce.
"""

import numpy as np
import ml_dtypes

import concourse.bass as bass
import concourse.tile as tile
from concourse import bacc, mybir
from concourse.bass_utils import run_bass_kernel_spmd

B, L, H, V = 16, 128, 1024, 50257
NC = 8
VC = 6656                # per-core vocab shard (13*512); 8*VC = 53248 >= V
HC = H // NC             # per-core hidden-dim shard (128)
G3 = 3 * H               # gate rows (r,z,n)
NT = G3 // 128           # 24 j-tiles of 128
NG = VC // 512           # 13 projection column groups of 512
NEG = -1.0e30

WSCALE = 512.0           # proj_w scaled by 2^9 so fp8e4 values are normal
HSCALE = 16.0            # h1 scaled by 2^4 before fp8e4 cast
OSCALE = 1.0 / (WSCALE * HSCALE)   # logits de-scale: 2^-13

f32 = mybir.dt.float32
bf16 = mybir.dt.bfloat16
f8e4 = mybir.dt.float8e4
FX = mybir.ActivationFunctionType
AX = mybir.AxisListType
ALU = mybir.AluOpType
DR = mybir.MatmulPerfMode.DoubleRow

NP_F8E4 = ml_dtypes.float8_e4m3
NP_BF16 = ml_dtypes.bfloat16

# blob column layout (f32, [128, 74]):
#   0:16 x0T | 16:40 biT | 40:64 bhT | 64 bw | 65 bb | 66:74 msk
BLOB_C = 74

# AllReduce payload tiles (each [128, tile, B] bf16):
#   0:16 gi+gh partials for r,z | 16:24 in_ | 24:32 hn | 32:40 h0 masked
AR_T = 40

# tail split: vector handles the first TS_V cols, scalar engine the rest
TS_V = 8 * 512

LAST_RESULT = None  # test harness reads profiling info from here
_NC_CACHE = None


def _bc(ap, insert_at, step, count):
    """Insert a broadcast/strided dim into an AP at position insert_at."""
    new = list(ap.ap)
    new.insert(insert_at, [step, count])
    return bass.AP(tensor=ap.tensor, offset=ap.offset, ap=new)


def _build():
    nc = bacc.Bacc("TRN2", target_bir_lowering=False, debug=False, num_devices=NC)

    blob = nc.dram_tensor("blob", [128, BLOB_C], f32, kind="ExternalInput").ap()
    # hid carries [hid | wihT | whhT] packed as one [128, 8192] bf16 tensor
    hid = nc.dram_tensor("hid", [L, 2048 + 2 * G3], bf16, kind="ExternalInput").ap()
    pw8 = nc.dram_tensor("pw8", [128, NG * 8 * 512], f8e4, kind="ExternalInput").ap()
    pb = nc.dram_tensor("pb", [1, VC], f32, kind="ExternalInput").ap()
    logp = nc.dram_tensor("logp", [B, VC], bf16, kind="ExternalOutput").ap()

    with tile.TileContext(nc) as tc:
        with (
            tc.tile_pool(name="singles", bufs=1) as singles,
            tc.tile_pool(name="gru_ps", bufs=1, space="PSUM") as gru_ps,
            tc.tile_pool(name="proj_ps", bufs=3, space="PSUM") as proj_ps,
            tc.tile_pool(name="gs_ps", bufs=1, space="PSUM") as gs_pool,
            tc.tile_pool(name="stats", bufs=4) as stats,
            tc.tile_pool(name="dram", bufs=1, space="DRAM") as dram,
        ):
            # ---- dummy collective: wake the ncfw firmware early -----------
            # (gathers garbage DRAM; output unused — only the barrier and
            # firmware warm-up matter)
            wcc_in = dram.tile([1, 8], f32, tag="wcc_in")
            wcc_out = dram.tile([NC, 8], f32, tag="wcc_out")
            nc.gpsimd.collective_compute(
                "AllGather", ALU.bypass,
                replica_groups=[list(range(NC))],
                ins=[wcc_in.opt()], outs=[wcc_out.opt()],
            )

            # ---- critical loads first on the sync HWDGE ring --------------
            # one [128, 8192] bf16 transfer carries hid | wihT | whhT; views
            # are hand-built APs into it (one DMA fixed cost instead of 3)
            ebuf = singles.tile([128, 2048 + 2 * G3], bf16, tag="ebuf")
            nc.sync.dma_start(out=ebuf, in_=hid)
            hid_sb = ebuf[:, 0:2048].rearrange("p (b h) -> p b h", b=B)
            wih_sb = ebuf[:, 2048 : 2048 + G3]
            whh_sb = ebuf[:, 2048 + G3 : 2048 + 2 * G3]

            # small/noncritical on the scalar ring
            blob_sb = singles.tile([128, BLOB_C], f32, tag="blob_sb")
            nc.scalar.dma_start(out=blob_sb, in_=blob)
            pbb = singles.tile([B, VC], f32, tag="pbb")
            nc.scalar.dma_start(out=pbb, in_=_bc(pb[0], 0, 0, B))

            # ---- fp8 weight stream (sync ring, after the critical loads;
            # 13 moderate transfers keep SDMA packet turns short) ----------
            # host layout per partition: [g(13), kc2(4), sub(2), j(2), c(256)]
            pw_sb = singles.tile([128, NG, 4, 2, 2, 256], f8e4, tag="pw_sb")
            pw_view = pw8.rearrange("p (g x) -> p g x", g=NG)
            for g in range(NG):
                nc.sync.dma_start(out=pw_sb[:, g], in_=pw_view[:, g])

            # ---- unpack blob ----------------------------------------------
            x0T_sb = singles.tile([HC, B], bf16, tag="x0T_sb")
            nc.scalar.activation(out=x0T_sb[:], in_=blob_sb[:, 0:16], func=FX.Relu)
            bw_sb = singles.tile([L, 1], bf16, tag="bw_sb")
            nc.vector.tensor_copy(bw_sb[:], blob_sb[:, 64:65])
            msk_sb = singles.tile([128, NC], bf16, tag="msk_sb")
            nc.vector.tensor_copy(msk_sb[:], blob_sb[:, 66:74])
            bsum = singles.tile([128, 16], f32, tag="bsum")
            nc.vector.tensor_add(bsum, blob_sb[:, 16:32], blob_sb[:, 40:56])

            # ---- bridge: h0T_c[h,b] = sum_l hidden[b,l,h]*w[l] ------------
            h0T_ps = gru_ps.tile([HC, B], f32, tag="h0T_ps")
            for b in range(B):
                nc.tensor.matmul(
                    h0T_ps[:, b : b + 1], hid_sb[:, b, :], bw_sb[:],
                    start=True, stop=True,
                )
            h0T_sb = singles.tile([HC, B], bf16, tag="h0T_sb")
            nc.vector.tensor_scalar_add(h0T_sb[:], h0T_ps[:], blob_sb[:, 65:66])

            # ---- partial gate pre-activations (T layout) ------------------
            # r,z gates: gi+gh accumulated in one PSUM group; n gate split.
            grz_ps = gru_ps.tile([128, 16, B], f32, tag="grz_ps")
            gin_ps = gru_ps.tile([128, 8, B], f32, tag="gin_ps")
            ghn_ps = gru_ps.tile([128, 8, B], f32, tag="ghn_ps")
            for t in range(16):
                nc.tensor.matmul(
                    grz_ps[:, t, :], wih_sb[:, t * 128 : (t + 1) * 128], x0T_sb[:],
                    start=True, stop=False,
                )
                nc.tensor.matmul(
                    grz_ps[:, t, :], whh_sb[:, t * 128 : (t + 1) * 128], h0T_sb[:],
                    start=False, stop=True,
                )
            for t in range(16, NT):
                nc.tensor.matmul(
                    gin_ps[:, t - 16, :], wih_sb[:, t * 128 : (t + 1) * 128], x0T_sb[:],
                    start=True, stop=True,
                )
                nc.tensor.matmul(
                    ghn_ps[:, t - 16, :], whh_sb[:, t * 128 : (t + 1) * 128], h0T_sb[:],
                    start=True, stop=True,
                )

            # ---- pack slim AllReduce payload [128, 40, 16] fp8e4 ----------
            arbuf = singles.tile([128, AR_T, B], f8e4, tag="arbuf")
            nc.vector.tensor_copy(arbuf[:, 0:16, :], grz_ps[:])
            nc.vector.tensor_copy(arbuf[:, 16:24, :], gin_ps[:])
            nc.vector.tensor_copy(arbuf[:, 24:32, :], ghn_ps[:])
            h0_bcast = _bc(h0T_sb[:], 1, 0, NC)          # [128, 8, 16]
            msk_bcast = _bc(msk_sb[:], 2, 0, B)          # [128, 8, 16]
            nc.vector.tensor_mul(arbuf[:, 32:40, :], h0_bcast, msk_bcast)

            # SWDGE path: completion tracked on its own sem lanes, not the
            # HWDGE lanes shared with the weight-stream DMAs
            cc_in = dram.tile([128, AR_T * B], f8e4, tag="cc_in")
            cc_out = dram.tile([128, AR_T * B], f8e4, tag="cc_out")
            nc.gpsimd.dma_start(out=cc_in[:], in_=arbuf[:])
            nc.gpsimd.collective_compute(
                "AllReduce", ALU.add,
                replica_groups=[list(range(NC))],
                ins=[cc_in.opt()], outs=[cc_out.opt()],
            )
            # preload sigmoid/tanh tables while the AllReduce is in flight
            tl = stats.tile([128, 1], f32, tag="tl")
            nc.scalar.activation(out=tl, in_=arbuf[:, 0:1, 0:1], func=FX.Tanh)
            nc.scalar.activation(out=tl, in_=tl, func=FX.Sigmoid)
            arx8 = singles.tile([128, AR_T, B], f8e4, tag="arx8")
            nc.scalar.dma_start(out=arx8[:], in_=cc_out[:])
            arx = singles.tile([128, AR_T, B], f32, tag="arx")
            nc.vector.tensor_copy(arx[:], arx8[:])

            # ---- gates (full width, every core redundantly) ---------------
            rT = singles.tile([128, NC, B], f32, tag="rT")
            nc.vector.tensor_add(rT[:], arx[:, 0:8, :], _bc(bsum[:, 0:8], 2, 0, B))
            nc.scalar.activation(out=rT[:], in_=rT[:], func=FX.Sigmoid)

            zT = singles.tile([128, NC, B], f32, tag="zT")
            nc.vector.tensor_add(zT[:], arx[:, 8:16, :], _bc(bsum[:, 8:16], 2, 0, B))
            nc.scalar.activation(out=zT[:], in_=zT[:], func=FX.Sigmoid)

            nT = singles.tile([128, NC, B], f32, tag="nT")
            nc.vector.tensor_add(nT[:], arx[:, 24:32, :], _bc(blob_sb[:, 56:64], 2, 0, B))
            nc.vector.tensor_mul(nT[:], nT[:], rT[:])
            nc.vector.tensor_add(nT[:], nT[:], arx[:, 16:24, :])
            nc.vector.tensor_add(nT[:], nT[:], _bc(blob_sb[:, 32:40], 2, 0, B))
            nc.scalar.activation(out=nT[:], in_=nT[:], func=FX.Tanh)

            h1T = singles.tile([128, NC, B], f32, tag="h1T")
            nc.vector.tensor_mul(h1T[:], zT[:], arx[:, 32:40, :])   # z*h0
            nc.vector.tensor_mul(zT[:], zT[:], nT[:])               # z*n
            nc.vector.tensor_add(h1T[:], h1T[:], nT[:])             # + n
            nc.vector.tensor_sub(h1T[:], h1T[:], zT[:])             # - z*n
            h1q = singles.tile([128, NC, B], f8e4, tag="h1q")
            nc.vector.tensor_scalar_mul(h1q[:], h1T[:], HSCALE)

            # ---- projection (DoubleRow fp8) + online sumexp ---------------
            logits_sb = singles.tile([B, VC], f32, tag="logits_sb")
            s_run = singles.tile([B, 1], f32, tag="s_run")
            nc.vector.memset(s_run, 0.0)

            for g in range(NG):
                col = g * 512
                lg = proj_ps.tile([B, 512], f32, tag="lg")
                for sub in range(2):
                    for k2 in range(4):
                        nc.tensor.matmul(
                            lg[:, sub * 256 : (sub + 1) * 256],
                            h1q[:, 2 * k2 : 2 * k2 + 2, :],
                            pw_sb[:, g, k2, sub],
                            start=(k2 == 0), stop=(k2 == 3),
                            perf_mode=DR,
                        )
                nc.vector.tensor_add(
                    logits_sb[:, col : col + 512], lg[:], pbb[:, col : col + 512]
                )
                # expb is a write-only sink (only the f32 accumulator csum is
                # consumed) — bf16 halves the ACT engine's write traffic
                expb = stats.tile([B, 512], bf16, tag="expb")
                csum = stats.tile([B, 1], f32, tag="csum")
                nc.scalar.activation(
                    out=expb[:], in_=logits_sb[:, col : col + 512], func=FX.Exp,
                    scale=OSCALE, accum_out=csum[:, 0:1],
                )
                nc.vector.tensor_add(s_run, s_run, csum)

            # preload Ln + Identity tables while the AllGather is in flight
            tl2 = stats.tile([B, 1], f32, tag="tl2")
            nc.scalar.activation(out=tl2, in_=s_run[:], func=FX.Ln)
            nc.scalar.activation(out=tl2, in_=tl2, func=FX.Identity, scale=1.0, bias=0.0)

            # ---- global sumexp (AllGather + ones-matmul reduce) -----------
            std_in = dram.tile([1, B], f32, tag="std_in")
            std_out = dram.tile([NC, B], f32, tag="std_out")
            nc.sync.dma_start(out=std_in[0:1, :], in_=s_run[:])
            nc.gpsimd.collective_compute(
                "AllGather", ALU.bypass,
                replica_groups=[list(range(NC))],
                ins=[std_in.opt()], outs=[std_out.opt()],
            )
            sall = singles.tile([NC, B], f32, tag="sall")
            nc.scalar.dma_start(out=sall, in_=std_out[:])
            ones8 = singles.tile([NC, 1], f32, tag="ones8")
            nc.vector.memset(ones8, 1.0)
            gS_ps = gs_pool.tile([B, 1], f32, tag="gS_ps")
            nc.tensor.matmul(gS_ps[:], sall[:], ones8[:], start=True, stop=True)
            ngS = singles.tile([B, 1], f32, tag="ngS")
            nc.scalar.activation(out=ngS, in_=gS_ps[:], func=FX.Ln)
            nc.vector.tensor_scalar_mul(ngS, ngS, -1.0)    # -lse

            # ---- logp = logits*OSCALE - lse, write out (split engines,
            # bf16 output halves writeback bytes; host upcasts) ------------
            lp16 = singles.tile([B, VC], bf16, tag="lp16")
            nc.vector.tensor_scalar(
                out=lp16[:, 0:TS_V], in0=logits_sb[:, 0:TS_V],
                scalar1=OSCALE, scalar2=ngS[:, 0:1], op0=ALU.mult, op1=ALU.add,
            )
            nc.scalar.activation(
                out=lp16[:, TS_V:VC], in_=logits_sb[:, TS_V:VC],
                func=FX.Identity, scale=OSCALE, bias=ngS[:, 0:1],
            )
            nc.sync.dma_start(out=logp[:, 0:TS_V], in_=lp16[:, 0:TS_V])
            nc.sync.dma_start(out=logp[:, TS_V:VC], in_=lp16[:, TS_V:VC])

    nc.compile()
    return nc


def kernel(input, hidden, emb, bridge_w, bridge_b, w_ih, w_hh, b_ih, b_hh,
           proj_w, proj_b):
    global _NC_CACHE, LAST_RESULT
    if _NC_CACHE is None:
        _NC_CACHE = _build()
    nc = _NC_CACHE

    input = np.asarray(input)
    hidden = np.asarray(hidden, dtype=np.float32)
    emb = np.asarray(emb, dtype=np.float32)
    bridge_w = np.asarray(bridge_w, dtype=np.float32)
    bridge_b = np.asarray(bridge_b, dtype=np.float32)
    w_ih = np.asarray(w_ih, dtype=np.float32)
    w_hh = np.asarray(w_hh, dtype=np.float32)
    b_ih = np.asarray(b_ih, dtype=np.float32)
    b_hh = np.asarray(b_hh, dtype=np.float32)
    proj_w = np.asarray(proj_w, dtype=np.float32)
    proj_b = np.asarray(proj_b, dtype=np.float32)

    x0 = emb[input[:, 0].astype(np.int64)]          # [B, H]
    x0T = np.ascontiguousarray(x0.T)                # [H, B] f32 (relu on device)
    hid_t = np.ascontiguousarray(hidden.transpose(1, 0, 2)).astype(NP_BF16)

    biT = np.ascontiguousarray(b_ih.reshape(NT, 128).T)   # [128, 24]
    bhT = np.ascontiguousarray(b_hh.reshape(NT, 128).T)

    in_maps = []
    for c in range(NC):
        hs = slice(c * HC, (c + 1) * HC)
        lo, hi = c * VC, min((c + 1) * VC, V)
        pw_blk = proj_w[lo:hi]
        pb_blk = proj_b[lo:hi]
        if hi - lo < VC:
            pad = VC - (hi - lo)
            pw_blk = np.concatenate([pw_blk, np.zeros((pad, H), np.float32)], axis=0)
            pb_blk = np.concatenate([pb_blk, np.full((pad,), NEG, np.float32)])
        # DoubleRow layout: h = kc2*256 + j*128 + p ; v = g*512 + sub*256 + cc
        pwT = np.ascontiguousarray(pw_blk.T) * WSCALE          # [H, VC]
        pw_i = pwT.reshape(4, 2, 128, NG, 2, 256)              # [kc2,j,p,g,sub,c]
        pw_i = pw_i.transpose(2, 3, 0, 4, 1, 5)                # [p,g,kc2,sub,j,c]
        pw_i = np.ascontiguousarray(pw_i).reshape(128, NG * 8 * 256 * 2 // 512 * 512)

        blob = np.zeros((128, BLOB_C), np.float32)
        blob[:, 0:16] = x0T[hs]
        blob[:, 16:40] = biT
        blob[:, 40:64] = bhT
        blob[:, 64] = bridge_w[0]
        blob[:, 65] = bridge_b[0]
        blob[:, 66 + c] = 1.0                                  # mask one-hot

        ebuf = np.concatenate([
            hid_t[:, :, hs].reshape(L, B * HC).astype(NP_BF16),
            np.ascontiguousarray(w_ih[:, hs].T).astype(NP_BF16),
            np.ascontiguousarray(w_hh[:, hs].T).astype(NP_BF16),
        ], axis=1)
        in_maps.append({
            "blob": blob,
            "hid": np.ascontiguousarray(ebuf),
            "pw8": pw_i.astype(NP_F8E4),
            "pb": np.ascontiguousarray((pb_blk * WSCALE).reshape(1, VC)),
        })

    res = run_bass_kernel_spmd(nc, in_maps, list(range(NC)))
    LAST_RESULT = res

    logp_full = np.concatenate(
        [res.results[c]["logp"].astype(np.float32) for c in range(NC)], axis=1
    )
    logp_full = np.ascontiguousarray(logp_full[:, :V])
    return np.broadcast_to(logp_full[:, None, :], (B, L - 1, V))
